# revision 39
# baseline (speedup 1.0000x reference)
"""EGAT kernel v2 for 8 Trainium2 NeuronCores.

Edge-parallel windowed design: edges sorted by dst, 8 dst-disjoint shards,
~50 windows/core of 2048 edge slots spanning <=128 dst rows.  Per window:
project gathered src/dst features to logits and payload, softmax weights
via exp of the attn dot, weight the payload, and aggregate per dst row
with a one-hot scatter matmul.  v2 rebalance vs the v1.5 baseline:

- dst-side logit projection is one fp8 DoubleRow matmul per tile (split-K
  over channel halves): 32 cyc/tile vs 64.  dfe tiles of consecutive
  window pairs share one full-width DMA ([ceil(W/2), 128, 2, slots];
  even window on partitions 0:64, odd on 64:128, wnj2 duplicated per
  half) -- the DMA cost model charges per-partition bytes, so a
  64-partition transfer would pay double.
- relu+attn fused into one DVE scalar_tensor_tensor per half window
  (replaces the ACT relu cross + Pool attn multiply).
- the 0.01-slope lin term of leaky_relu is dropped: its logit
  contribution sits below bf16 rounding of eat (verified vs reference).
- attn-dot reduce of 16 runs as a 4-level add tree (lvl1-2 Pool,
  lvl3-4 Pool) instead of one DVE tensor_reduce.
- one merged exp per window; its [128,t,4] bf16 output feeds the scatter
  weight columns (copied into rhp) and the payload multiplies directly.
- payload PSUM crossing in 8 chunks of 2 tiles: per-chunk engine
  pattern "aavaavva" (a = ACT-stage + Pool-mult, v = DVE fused mult
  from PSUM).
- scatter is a single 260-col matmul per tile (payload + weight sums).
- epilogue: ACT crosses P once; sg/recip on DVE in f32; si-multiply on
  Pool; head-mean reduce + batched bias/relu on DVE; output DMA per 3
  windows into a [128, W, 64] layout.
- bf16 consts ride one [128, 512] DMA; all DMAs on the SP queue.

PSUM banks: logit pool 3 bufs, payload-chunk pool 3, scatter pool 2
(the third logit buffer relieves the prL-recycle stall on the PE).

Cost-model estimate 218174 ns/core (baseline 282281); HW-verified
rel err 0.0051 (gate 2e-2).
"""

import sys

sys.path.insert(0, "/opt/trn_rl_repo")

import numpy as np
import ml_dtypes

BF16 = ml_dtypes.bfloat16
FP8 = ml_dtypes.float8_e4m3

# ---- problem constants (hardcoded per the task contract) -------------------
N_SRC = 50000
N_DST = 50000
E = 800000
IN_NODE = 128
IN_EDGE = 16
OUT_NODE = 64
OUT_EDGE = 16
H = 4
SLOPE = 0.01

N_CORES = 8

FE = H * OUT_EDGE            # 64 logit cols
NPAY = H * OUT_NODE          # 256 payload cols
NPROJ = FE + NPAY            # 320
PW = NPAY + H                # 260 scatter cols (payload + 4 weight sums)
LSCALE = 8.0                 # logit-projection scale (fp8 subnormal dodge)


def default_cfg():
    return dict(
        n_dst=N_DST,
        t_half=8,             # tiles per half-window (8 -> 1024 slots/half)
        span=128,             # max dst rows per window
        use_dr=True,          # fp8 DoubleRow for dst-side logit matmuls
        ohw_q="sync",         # DMA queue for the one-hot
        pb_eng="scalar",      # engine for the P psum->sbuf cross
        tree34="gpsimd",      # engine for eat-tree levels 3-4
        prio=300,             # priority boost for P_ext-releasing ops
        tch=2,                # tiles per payload psum chunk
        chasg="aavaavva",     # per-chunk engine: a=ACT+Pool, v=DVE-fused
        out_q="sync",         # DMA queue for the output
        ob=3,                 # windows per output DMA batch
        exp_half=False,       # one merged exp per window
        w0_prio=False,        # priority boost for window-0 DMAs
        cq="sync",            # DMA queue for one-time consts
        tailv=0,              # trailing windows with all-DVE chunks
        psl=3,                # PSUM bufs for the logit pool
        pspay=3,              # PSUM bufs for the payload-chunk pool
    )


# ===========================================================================
# Host-side packing
# ===========================================================================

def prep(nfeats, dst_feats, reward, src, dst, W_ns, b_ns, W_ni, W_nj, W_fij,
         attn, b_e, cfg=None):
    """Sort/shard/pack everything. Returns (meta, in_maps)."""
    cfg = cfg or default_cfg()
    n_dst = cfg["n_dst"]
    t_half = cfg["t_half"]
    span = cfg["span"]
    slots = 2 * t_half * 128    # slots per window
    t_w = 2 * t_half

    e_tot = src.shape[0]

    nfeats = np.asarray(nfeats, np.float32)
    dst_feats = np.asarray(dst_feats, np.float32)
    reward = np.asarray(reward, np.float32)
    src = np.asarray(src, np.int64)
    dst = np.asarray(dst, np.int64)
    W_ns = np.asarray(W_ns, np.float32)
    b_ns = np.asarray(b_ns, np.float32)
    W_ni = np.asarray(W_ni, np.float32)
    W_nj = np.asarray(W_nj, np.float32)
    W_fij = np.asarray(W_fij, np.float32)
    attn = np.asarray(attn, np.float32)
    b_e = np.asarray(b_e, np.float32)

    # ---- sort by dst and shard at dst boundaries --------------------------
    order = np.argsort(dst, kind="stable")
    d_s = dst[order]
    s_s = src[order]
    r_s = reward[order]

    cut = [0]
    for c in range(1, N_CORES):
        t = (e_tot * c) // N_CORES
        while t < e_tot and t > 0 and d_s[t] == d_s[t - 1]:
            t += 1
        cut.append(t)
    cut.append(e_tot)

    # ---- greedy window packing per core -----------------------------------
    per_core = []
    for c in range(N_CORES):
        e0, e1 = cut[c], cut[c + 1]
        d = d_s[e0:e1]
        wins = []  # (base, w_start, w_count) over local positions
        if e1 > e0:
            uniq, starts = np.unique(d, return_index=True)
            ends = np.append(starts[1:], len(d))
            base = None
            w_start = 0
            w_count = 0
            for gi in range(len(uniq)):
                dd = int(uniq[gi])
                glen = int(ends[gi] - starts[gi])
                if (base is None or dd - base > span - 1
                        or w_count + glen > slots):
                    if base is not None:
                        wins.append((base, w_start, w_count))
                    base = dd
                    w_start = int(starts[gi])
                    w_count = 0
                w_count += glen
            wins.append((base, w_start, w_count))
        per_core.append((e0, e1, wins))

    W = max(1, max(len(pc[2]) for pc in per_core))

    # virtual feature rows: x_row @ W_nj == colsum(W_fij); y_row @ W_nj == b_e
    wsum = W_fij.sum(axis=0)
    x_row = np.linalg.lstsq(W_nj.T.astype(np.float64), wsum.astype(np.float64),
                            rcond=None)[0].astype(np.float32)
    y_row = np.linalg.lstsq(W_nj.T.astype(np.float64), b_e.astype(np.float64),
                            rcond=None)[0].astype(np.float32)

    mf_all = []     # [128, W, t_w, 128] fp8 one-hot per slot
    zfe_all = []    # per core (zfe bf16 [128, W*slots], dfe8 [64, 2, W*slots])
    asm = []        # per core (slot_rows, global_rows)

    for c in range(N_CORES):
        e0, e1, wins = per_core[c]
        d = d_s[e0:e1]
        s = s_s[e0:e1]
        r = r_s[e0:e1]

        drel = np.full((W, slots), -1.0, np.float32)
        nfe = np.zeros((W * slots, IN_NODE), np.float32)
        dfe = np.zeros((W * slots, IN_NODE), np.float32)
        rows_slot = []
        rows_glob = []
        for w, (base, ws, wc) in enumerate(wins):
            sl = slice(ws, ws + wc)
            drel[w, :wc] = (d[sl] - base).astype(np.float32)
            nfe[w * slots:w * slots + wc] = nfeats[s[sl]]
            dfe[w * slots:w * slots + wc] = (dst_feats[d[sl]]
                                             + r[sl, None] * x_row[None, :]
                                             + y_row[None, :])
            uds = np.unique(d[sl])
            rows_slot.append(w * 128 + (uds - base))
            rows_glob.append(uds)

        # one-hot per slot, layout [128 p, W, t, 128 dcol]
        ohm = (drel.reshape(W, t_w, 128)[:, :, :, None]
               == np.arange(128, dtype=np.float32)).astype(FP8)
        ohm = np.ascontiguousarray(ohm.transpose(2, 0, 1, 3))

        zfe = np.ascontiguousarray(
            nfe.T.reshape(IN_NODE, W * slots).astype(BF16))
        if cfg.get("use_dr", True):
            # dfe channel-split for DoubleRow, window-pair packed:
            # [ceil(W/2), 128, 2, slots] -- even window on partitions 0:64,
            # odd window on 64:128, so each DMA moves a full-width tile.
            d4 = dfe.reshape(W, slots, 2, 64).transpose(0, 3, 2, 1).astype(FP8)
            if W % 2:
                d4 = np.concatenate(
                    [d4, np.zeros_like(d4[:1])], axis=0)
            df8 = np.ascontiguousarray(
                np.concatenate([d4[0::2], d4[1::2]], axis=1))
        else:
            df8 = np.ascontiguousarray(
                dfe.T.reshape(IN_NODE, W * slots).astype(FP8))
        mf_all.append(ohm)
        zfe_all.append((zfe, df8))
        asm.append((np.concatenate(rows_slot) if rows_slot else
                    np.zeros(0, np.int64),
                    np.concatenate(rows_glob) if rows_glob else
                    np.zeros(0, np.int64)))

    # ---- shared constants -------------------------------------------------
    # e = sum_f attn*leaky(x) ~= 0.99*sum_f attn*relu(x)  (relu
    # decomposition of leaky_relu; the 0.99 is folded into attn_rep and
    # the 0.01-slope lin term is dropped -- its logit contribution sits
    # below bf16 rounding of eat, verified vs the reference).
    wcat = np.concatenate([W_ni * LSCALE, W_ns * 0.25],
                          axis=1).astype(BF16)                       # [128,320]
    wnj_full = W_nj * LSCALE
    if cfg.get("use_dr", True):
        w2 = wnj_full.reshape(2, 64, FE).transpose(1, 0, 2).astype(FP8)
        wnj2 = np.ascontiguousarray(np.concatenate([w2, w2], axis=0))
    else:
        wnj2 = np.ascontiguousarray(wnj_full.astype(FP8))
    attn_rep = np.broadcast_to(
        (0.99 / LSCALE) * attn.reshape(-1).astype(np.float32),
        (128, H * OUT_EDGE)).astype(BF16).copy()
    bmean = np.broadcast_to(b_ns.reshape(H, OUT_NODE).mean(axis=0),
                            (128, OUT_NODE)).astype(BF16).copy()
    negb = (-bmean.astype(np.float32)).astype(BF16)

    consts = np.concatenate(
        [wcat, attn_rep, bmean, negb], axis=1)                   # [128, 512]
    in_maps = []
    for c in range(N_CORES):
        in_maps.append(dict(
            zfe=zfe_all[c][0], dfe8=zfe_all[c][1], ohm=mf_all[c],
            consts=consts, wnj2=wnj2,
        ))

    meta = dict(W=W, asm=asm, cfg=cfg)
    return meta, in_maps


# ===========================================================================
# Device program
# ===========================================================================

def build_program(W, cfg, fused_chunks=None):
    import concourse.bacc as bacc
    import concourse.tile as tile
    import concourse.mybir as mybir
    from contextlib import ExitStack

    dt = mybir.dt
    AF = mybir.ActivationFunctionType
    OP = mybir.AluOpType
    PM = mybir.MatmulPerfMode

    if fused_chunks is None:
        fused_chunks = tuple(cfg.get("fused", (1,)))
    t_half = cfg["t_half"]
    t_w = 2 * t_half
    slots = t_w * 128
    TCH = cfg.get("tch", 4)      # tiles per payload psum chunk
    n_ch = t_w // TCH            # payload chunks per window
    OB = cfg.get("ob", 4)       # windows per output DMA batch

    nc = bacc.Bacc(None, target_bir_lowering=False)

    ZFE = nc.declare_dram_parameter("zfe", [IN_NODE, W * slots],
                                    dt.bfloat16, isOutput=False)
    use_dr = cfg.get("use_dr", True)
    if use_dr:
        DFE8 = nc.declare_dram_parameter("dfe8",
                                         [(W + 1) // 2, 128, 2, slots],
                                         dt.float8e4, isOutput=False)
        WNJ2 = nc.declare_dram_parameter("wnj2", [128, 2, FE], dt.float8e4,
                                         isOutput=False)
    else:
        DFE8 = nc.declare_dram_parameter("dfe8", [IN_NODE, W * slots],
                                         dt.float8e4, isOutput=False)
        WNJ2 = nc.declare_dram_parameter("wnj2", [IN_NODE, FE],
                                         dt.float8e4, isOutput=False)
    OHM = nc.declare_dram_parameter("ohm", [128, W, t_w, 128],
                                    dt.float8e4, isOutput=False)
    CONSTS = nc.declare_dram_parameter("consts", [IN_NODE, NPROJ + 3 * FE],
                                       dt.bfloat16, isOutput=False)
    OUT = nc.declare_dram_parameter("out", [128, W, OUT_NODE], dt.bfloat16,
                                    isOutput=True)

    with tile.TileContext(nc) as tc, ExitStack() as ctx:
        cpool = ctx.enter_context(tc.tile_pool(name="consts", bufs=1))
        call_s = cpool.tile([128, NPROJ + 3 * FE], dt.bfloat16)
        getattr(nc, cfg.get("cq", "scalar")).dma_start(call_s[:], CONSTS[:])
        wcat_s = call_s[:, 0:NPROJ]
        attn_s = call_s[:, NPROJ:NPROJ + FE]
        bmean_s = call_s[:, NPROJ + FE:NPROJ + FE + OUT_NODE]
        negb_s = call_s[:, NPROJ + FE + OUT_NODE:NPROJ + FE + 2 * OUT_NODE]
        if use_dr:
            wnj2_s = cpool.tile([128, 2, FE], dt.float8e4)
            getattr(nc, cfg.get("cq", "scalar")).dma_start(wnj2_s[:], WNJ2[:])
        else:
            wnjf_s = cpool.tile([128, FE], dt.float8e4)
            nc.sync.dma_start(wnjf_s[:], WNJ2[:])

        BUFS = cfg.get("bufs", (5, 5, 5, 8, 4, 6))
        with tc.tile_pool(name="feat", bufs=BUFS[0]) as fpool, \
             tc.tile_pool(name="meta", bufs=BUFS[1]) as mpool, \
             tc.tile_pool(name="lrp", bufs=BUFS[2]) as lpool, \
             tc.tile_pool(name="stgp", bufs=BUFS[3]) as spool, \
             tc.tile_pool(name="rhsp", bufs=BUFS[4]) as rpool, \
             tc.tile_pool(name="ep", bufs=BUFS[5]) as epool, \
             tc.tile_pool(name="psL", bufs=cfg.get("psl", 2),
                          space="PSUM") as psL, \
             tc.tile_pool(name="psPay", bufs=cfg.get("pspay", 8 // TCH),
                          space="PSUM") as psPay, \
             tc.tile_pool(name="psP", bufs=cfg.get("psp", 2),
                          space="PSUM") as psP:
            for w in range(W):
                import contextlib
                prio_ctx = (tc.high_priority()
                            if w == 0 and cfg.get("w0_prio", True)
                            else contextlib.nullcontext())
                with prio_ctx:
                    nfe = fpool.tile([128, slots], dt.bfloat16, tag="nfe")
                    nq = (nc.scalar if (w == 0 and cfg.get("w0_nfe_sc", False))
                          else nc.sync)
                    if w == 0 and cfg.get("w0_split", False):
                        hs = slots // 2
                        nq.dma_start(nfe[:, 0:hs], ZFE[:, 0:hs])
                        nq.dma_start(nfe[:, hs:slots], ZFE[:, hs:slots])
                    else:
                        nq.dma_start(nfe[:],
                                     ZFE[:, w * slots:(w + 1) * slots])
                if use_dr:
                    if w % 2 == 0:
                        dq = (nc.scalar if (w == 0 and cfg.get("w0_dfe_sc",
                                                               False))
                              else nc.sync)
                        with (tc.high_priority()
                              if w == 0 and cfg.get("w0_prio", True)
                              else contextlib.nullcontext()):
                            dfep = fpool.tile([128, 2, slots], dt.float8e4,
                                              tag="dfe", name="dfep")
                            if w == 0 and cfg.get("w0_split", False):
                                hs = slots // 2
                                dq.dma_start(dfep[:, :, 0:hs],
                                             DFE8[0, :, :, 0:hs])
                                dq.dma_start(dfep[:, :, hs:slots],
                                             DFE8[0, :, :, hs:slots])
                            else:
                                dq.dma_start(dfep[:], DFE8[w // 2, :, :, :])
                    p0 = 64 * (w % 2)
                    dfe = dfep[p0:p0 + 64, :, :]
                    wnjv = wnj2_s[p0:p0 + 64, :, :]
                else:
                    dfe = fpool.tile([128, slots], dt.float8e4, tag="dfe")
                    nc.sync.dma_start(dfe[:],
                                      DFE8[:, w * slots:(w + 1) * slots])
                ohw = mpool.tile([128, t_w, 128], dt.float8e4, tag="ohw")
                getattr(nc, cfg.get("ohw_q", "sync")).dma_start(
                    ohw[:], OHM[:, w, :, :])

                P = psP.tile([128, PW], dt.float32, tag="P")
                rhp = rpool.tile([128, t_w, PW], dt.bfloat16, tag="rhp")
                eat = lpool.tile([128, t_w, H], dt.bfloat16, tag="eat")
                wes = lpool.tile([128, t_w, H], dt.bfloat16, tag="wes")

                prPs = {}
                for hf in range(2):
                    ts0 = hf * t_half

                    def emit_pay(ts0=ts0, hf=hf):
                        for ci in range(t_half // TCH):
                            ch = hf * (t_half // TCH) + ci
                            prP = psPay.tile([128, TCH, NPAY], dt.float32,
                                             tag="prP", name="prP")
                            for k in range(TCH):
                                tl = ci * TCH + k
                                c0 = (ts0 + tl) * 128
                                nc.tensor.matmul(
                                    prP[:, k, :], lhsT=nfe[:, c0:c0 + 128],
                                    rhs=call_s[:, FE:NPROJ], start=True,
                                    stop=True, skip_group_check=True)
                            prPs[ch] = prP

                    if w == 0 and cfg.get("w0_payfirst", False):
                        emit_pay()
                    prL = psL.tile([128, t_half, FE], dt.float32, tag="prL")
                    for tl in range(t_half):
                        t = ts0 + tl
                        c0 = t * 128
                        nc.tensor.matmul(
                            prL[:, tl, :], lhsT=nfe[:, c0:c0 + 128],
                            rhs=call_s[:, 0:FE], start=True, stop=False,
                            skip_group_check=True)
                        if use_dr:
                            nc.tensor.matmul(
                                prL[:, tl, :], lhsT=dfe[:, :, c0:c0 + 128],
                                rhs=wnjv, start=False, stop=True,
                                perf_mode=PM.DoubleRow, skip_group_check=True)
                        else:
                            nc.tensor.matmul(
                                prL[:, tl, :], lhsT=dfe[:, c0:c0 + 128],
                                rhs=wnjf_s[:, 0:FE], start=False, stop=True,
                                skip_group_check=True)
                    # fused relu+attn on DVE (reads PSUM), then head reduce
                    ea = lpool.tile([128, t_half, FE], dt.bfloat16, tag="ea")
                    import contextlib as _ctl
                    with (tc.high_priority(offset=cfg.get("stt_prio", 0))
                          if cfg.get("stt_prio", 0) else _ctl.nullcontext()):
                        nc.vector.scalar_tensor_tensor(
                            out=ea[:], in0=prL[:], scalar=0.0,
                            in1=attn_s.unsqueeze(1).broadcast_to(
                                [128, t_half, FE]),
                            op0=OP.max, op1=OP.mult)
                    # attn-dot reduce of 16 as a 4-level add tree:
                    # lvl1-2 on Pool, lvl3-4 on DVE (2x-capable widths)
                    eav = ea[:].rearrange("p t (h f) -> p t h f", f=OUT_EDGE)
                    tr1 = lpool.tile([128, t_half, H, 8], dt.bfloat16,
                                     tag="tr1")
                    nc.gpsimd.tensor_tensor(out=tr1[:], in0=eav[:, :, :, 0:8],
                                            in1=eav[:, :, :, 8:16], op=OP.add)
                    tr2 = lpool.tile([128, t_half, H, 4], dt.bfloat16,
                                     tag="tr2")
                    nc.gpsimd.tensor_tensor(out=tr2[:], in0=tr1[:, :, :, 0:4],
                                            in1=tr1[:, :, :, 4:8], op=OP.add)
                    tr3 = lpool.tile([128, t_half, H, 2], dt.bfloat16,
                                     tag="tr3")
                    e34 = getattr(nc, cfg.get("tree34", "gpsimd"))
                    e34.tensor_tensor(out=tr3[:], in0=tr2[:, :, :, 0:2],
                                      in1=tr2[:, :, :, 2:4], op=OP.add)
                    e34.tensor_tensor(
                        out=eat[:, ts0:ts0 + t_half, :],
                        in0=tr3[:, :, :, 0], in1=tr3[:, :, :, 1], op=OP.add)
                    if cfg.get("exp_half", True) or (
                            w >= W - cfg.get("tail_eh", 0)):
                        nc.scalar.activation(wes[:, ts0:ts0 + t_half, :],
                                             eat[:, ts0:ts0 + t_half, :],
                                             AF.Exp)
                    if not (w == 0 and cfg.get("w0_payfirst", False)):
                        emit_pay()

                # ---- weights: we = exp(eat) -------------------------------
                if not (cfg.get("exp_half", True)
                        or w >= W - cfg.get("tail_eh", 0)):
                    nc.scalar.activation(wes[:], eat[:], AF.Exp)

                # ---- payload x weight per chunk ---------------------------
                nc.gpsimd.tensor_scalar(
                    out=rhp[:, :, NPAY:PW], in0=wes[:],
                    scalar1=1.0, scalar2=None, op0=OP.mult)
                chasg = cfg.get("chasg", None)
                if chasg and w >= W - cfg.get("tailv", 0):
                    chasg = "v" * n_ch
                for ch in range(n_ch):
                    t0 = ch * TCH
                    w_rep = (wes[:, t0:t0 + TCH, :].unsqueeze(3)
                             .broadcast_to([128, TCH, H, OUT_NODE]))
                    out_v = (rhp[:, t0:t0 + TCH, 0:NPAY]
                             .rearrange("p t (h f) -> p t h f", f=OUT_NODE))
                    kind = (chasg[ch] if chasg
                            else ("v" if ch in fused_chunks else "a"))
                    if kind == "v":
                        # fused crossing+mult from psum on DVE
                        nc.vector.tensor_tensor(
                            out=out_v,
                            in0=prPs[ch][:, :, :].rearrange(
                                "p t (h f) -> p t h f", f=OUT_NODE),
                            in1=w_rep, op=OP.mult)
                    else:
                        stg = spool.tile([128, TCH, NPAY], dt.bfloat16,
                                         tag="stg")
                        nc.scalar.copy(stg[:], prPs[ch][:, :, :])
                        eng = nc.gpsimd if kind == "a" else nc.vector
                        eng.tensor_tensor(
                            out=out_v,
                            in0=stg[:].rearrange("p t (h f) -> p t h f",
                                                 f=OUT_NODE),
                            in1=w_rep, op=OP.mult)

                # ---- scatter ---------------------------------------------
                for t in range(t_w):
                    nc.tensor.matmul(P[:], lhsT=ohw[:, t, :],
                                     rhs=rhp[:, t, :],
                                     start=(t == 0), stop=(t == t_w - 1),
                                     skip_group_check=True)

                # ---- epilogue --------------------------------------------
                pb = epool.tile([128, PW], dt.bfloat16, tag="pb")
                with tc.high_priority(offset=cfg.get("prio", 300)):
                    if cfg.get("pb_eng", "vector") == "vector":
                        nc.vector.tensor_scalar(
                            out=pb[:], in0=P[:], scalar1=0.0, scalar2=None,
                            op0=OP.add)
                    else:
                        nc.scalar.copy(pb[:], P[:])
                sg = epool.tile([128, H, 1], dt.float32, tag="sg")
                with tc.high_priority(offset=cfg.get("prio", 300)):
                    nc.vector.tensor_scalar(
                        out=sg[:],
                        in0=P[:, NPAY:PW].rearrange("p (h b) -> p h b", b=1),
                        scalar1=1e-30, scalar2=None, op0=OP.max)
                si = epool.tile([128, H, 1], dt.float32, tag="si")
                nc.vector.reciprocal(si[:], sg[:])
                tmp = epool.tile([128, H, OUT_NODE], dt.bfloat16, tag="tmp")
                nc.gpsimd.tensor_tensor(
                    out=tmp[:],
                    in0=pb[:, 0:NPAY].rearrange("p (h f) -> p h f",
                                                f=OUT_NODE),
                    in1=si[:].broadcast_to([128, H, OUT_NODE]),
                    op=OP.mult)
                ob = w % OB
                if ob == 0:
                    outf = epool.tile([128, OB, OUT_NODE], dt.bfloat16,
                                      tag="outf", name="outf")
                    outf_cur = outf
                with nc.allow_low_precision("4-term head mean; output is "
                                            "bf16 anyway"):
                    nc.vector.tensor_reduce(
                        outf_cur[:, ob, :].unsqueeze(2),
                        tmp[:].rearrange("p h f -> p f h"),
                        axis=mybir.AxisListType.X, op=OP.add)
                if ob == OB - 1 or w == W - 1:
                    # bias + relu:  relu(x + b) == max(x, -b) + b
                    nb = ob + 1
                    nc.vector.tensor_tensor(
                        out=outf_cur[:, 0:nb, :], in0=outf_cur[:, 0:nb, :],
                        in1=negb_s.unsqueeze(1).broadcast_to(
                            [128, nb, OUT_NODE]), op=OP.max)
                    nc.vector.tensor_tensor(
                        out=outf_cur[:, 0:nb, :], in0=outf_cur[:, 0:nb, :],
                        in1=bmean_s.unsqueeze(1).broadcast_to(
                            [128, nb, OUT_NODE]), op=OP.add)
                    w0 = w - ob
                    getattr(nc, cfg.get("out_q", "scalar")).dma_start(
                        OUT[:, w0:w + 1, :], outf_cur[:, 0:nb, :])

    if not nc.is_finalized():
        nc.finalize()
    return nc


# ===========================================================================
# numpy emulation of the device program (for validation/debug)
# ===========================================================================

def emulate_core(in_map, W, cfg):
    t_half = cfg["t_half"]
    slots = 2 * t_half * 128

    f32 = np.float32
    consts = in_map["consts"].astype(f32)
    wcat = consts[:, 0:NPROJ]
    wnj2 = in_map["wnj2"].astype(f32)
    if wnj2.ndim == 3:
        wnj = wnj2.transpose(1, 0, 2).reshape(IN_NODE, FE)
    else:
        wnj = wnj2
    attn_rep = consts[0, NPROJ:NPROJ + FE]
    bmean = consts[0, NPROJ + FE:NPROJ + FE + OUT_NODE]

    out = np.zeros((W * 128, OUT_NODE), f32)
    for w in range(W):
        nfe = in_map["zfe"][:, w * slots:(w + 1) * slots].astype(f32).T
        d8 = in_map["dfe8"]
        if d8.ndim == 4:
            p0 = 64 * (w % 2)
            dfe = (d8[w // 2, p0:p0 + 64].astype(f32).transpose(1, 0, 2)
                   .reshape(IN_NODE, slots).T)
        else:
            dfe = d8[:, w * slots:(w + 1) * slots].astype(f32).T
        pay = (nfe @ wcat[:, FE:NPROJ]).astype(BF16).astype(f32)
        fout = (nfe @ wcat[:, 0:FE] + dfe @ wnj[:, 0:FE])
        r = (np.maximum(fout, 0.0) * attn_rep[None, :]).astype(BF16).astype(f32)
        eat = (r.reshape(-1, H, OUT_EDGE).sum(axis=2)).astype(BF16).astype(f32)
        wgt = np.exp(eat).astype(BF16).astype(f32)            # [slots, H]
        oh = (in_map["ohm"][:, w].astype(f32).transpose(1, 0, 2)
              .reshape(slots, 128))
        rhs = np.concatenate(
            [(pay.reshape(-1, H, OUT_NODE)
              * wgt[:, :, None]).reshape(-1, NPAY).astype(BF16).astype(f32),
             wgt], axis=1)
        P = oh.T @ rhs                                        # [128, 260]
        pb = P.astype(BF16).astype(f32)
        s = np.maximum(P[:, NPAY:PW], 1e-30)
        tmp = (pb[:, 0:NPAY].reshape(128, H, OUT_NODE)
               / s[:, :, None]).astype(BF16).astype(f32)
        acc = tmp.sum(axis=1).astype(BF16).astype(f32)
        res = np.maximum(acc, -bmean[None, :]) + bmean[None, :]
        out[w * 128:(w + 1) * 128] = res.astype(BF16).astype(f32)
    return out


def assemble(meta, results):
    n_dst = meta["cfg"]["n_dst"]
    out = np.zeros((n_dst, OUT_NODE), np.float32)
    for c in range(N_CORES):
        slots_rows, glob_rows = meta["asm"][c]
        if len(glob_rows):
            flat = (results[c]["out"].astype(np.float32)
                    .transpose(1, 0, 2).reshape(-1, OUT_NODE))
            out[glob_rows] = flat[slots_rows]
    return out


# ===========================================================================
# entry point
# ===========================================================================

_CACHE = {}
LAST_EXEC_NS = None
LAST_RESULT = None


def kernel(nfeats, dst_feats, reward, src, dst,
           W_ns, b_ns, W_ni, W_nj, W_fij, attn, b_e):
    global LAST_EXEC_NS, LAST_RESULT
    import os
    from concourse.bass_utils import run_bass_kernel_spmd

    meta, in_maps = prep(nfeats, dst_feats, reward, src, dst,
                         W_ns, b_ns, W_ni, W_nj, W_fij, attn, b_e)
    key = meta["W"]
    if key not in _CACHE:
        _CACHE[key] = build_program(meta["W"], meta["cfg"])
    nc = _CACHE[key]
    kwargs = {}
    if os.environ.get("EGAT_TRACE"):
        kwargs = dict(trace=True)
    try:
        res = run_bass_kernel_spmd(nc, in_maps, list(range(N_CORES)), **kwargs)
    except ModuleNotFoundError:
        res = run_bass_kernel_spmd(nc, in_maps, list(range(N_CORES)))
    LAST_EXEC_NS = res.exec_time_ns
    LAST_RESULT = res
    return assemble(meta, res.results)


def estimate_ns(W=None, cfg=None):
    """Cost-model (no_exec CoreSim) estimate of the per-core kernel time.

    Always builds a fresh program: sharing an nc between CoreSim and a
    real run corrupts both (the run inflates the estimate, and a prior
    estimate breaks the subsequent compile).
    """
    from concourse.bass_interp import CoreSim
    cfg = cfg or default_cfg()
    if W is None:
        W = sorted(_CACHE)[0] if _CACHE else 50
    nc = build_program(W, cfg)
    sim = CoreSim(nc, no_exec=True, publish_trace=False)
    sim.simulate()
    return int(sim.time)


# revision 41
# speedup vs baseline: 1.0001x; 1.0001x over previous
"""EGAT kernel v2 for 8 Trainium2 NeuronCores.

Edge-parallel windowed design: edges sorted by dst, 8 dst-disjoint shards,
~50 windows/core of 2048 edge slots spanning <=128 dst rows.  Per window:
project gathered src/dst features to logits and payload, softmax weights
via exp of the attn dot, weight the payload, and aggregate per dst row
with a one-hot scatter matmul.  v2 rebalance vs the v1.5 baseline:

- dst-side logit projection is one fp8 DoubleRow matmul per tile (split-K
  over channel halves): 32 cyc/tile vs 64.  dfe tiles of consecutive
  window pairs share one full-width DMA ([ceil(W/2), 128, 2, slots];
  even window on partitions 0:64, odd on 64:128, wnj2 duplicated per
  half) -- the DMA cost model charges per-partition bytes, so a
  64-partition transfer would pay double.
- relu+attn fused into one DVE scalar_tensor_tensor per half window
  (replaces the ACT relu cross + Pool attn multiply).
- the 0.01-slope lin term of leaky_relu is dropped: its logit
  contribution sits below bf16 rounding of eat (verified vs reference).
- attn-dot reduce of 16 runs as a 4-level add tree (lvl1-2 Pool,
  lvl3-4 Pool) instead of one DVE tensor_reduce.
- one merged exp per window; its [128,t,4] bf16 output feeds the scatter
  weight columns (copied into rhp) and the payload multiplies directly.
- payload PSUM crossing in 8 chunks of 2 tiles: per-chunk engine
  pattern "aavaavva" (a = ACT-stage + Pool-mult, v = DVE fused mult
  from PSUM).
- scatter is a single 260-col matmul per tile (payload + weight sums).
- epilogue: ACT crosses P once; sg/recip on DVE in f32; si-multiply on
  Pool; head-mean reduce + batched bias/relu on DVE; output DMA per 3
  windows into a [128, W, 64] layout.
- bf16 consts ride one [128, 512] DMA; all DMAs on the SP queue.

PSUM banks: logit pool 3 bufs, payload-chunk pool 3, scatter pool 2
(the third logit buffer relieves the prL-recycle stall on the PE).

Cost-model estimate 218174 ns/core (baseline 282281); HW-verified
rel err 0.0051 (gate 2e-2).
"""

import sys

sys.path.insert(0, "/opt/trn_rl_repo")

import numpy as np
import ml_dtypes

BF16 = ml_dtypes.bfloat16
FP8 = ml_dtypes.float8_e4m3

# ---- problem constants (hardcoded per the task contract) -------------------
N_SRC = 50000
N_DST = 50000
E = 800000
IN_NODE = 128
IN_EDGE = 16
OUT_NODE = 64
OUT_EDGE = 16
H = 4
SLOPE = 0.01

N_CORES = 8

FE = H * OUT_EDGE            # 64 logit cols
NPAY = H * OUT_NODE          # 256 payload cols
NPROJ = FE + NPAY            # 320
PW = NPAY + H                # 260 scatter cols (payload + 4 weight sums)
LSCALE = 8.0                 # logit-projection scale (fp8 subnormal dodge)


def default_cfg():
    return dict(
        n_dst=N_DST,
        t_half=8,             # tiles per half-window (8 -> 1024 slots/half)
        span=128,             # max dst rows per window
        use_dr=True,          # fp8 DoubleRow for dst-side logit matmuls
        ohw_q="sync",         # DMA queue for the one-hot
        pb_eng="scalar",      # engine for the P psum->sbuf cross
        tree34="gpsimd",      # engine for eat-tree levels 3-4
        prio=300,             # priority boost for P_ext-releasing ops
        tch=2,                # tiles per payload psum chunk
        chasg="aavaavva",     # per-chunk engine: a=ACT+Pool, v=DVE-fused
        out_q="sync",         # DMA queue for the output
        ob=3,                 # windows per output DMA batch
        exp_half=False,       # one merged exp per window
        w0_prio=False,        # priority boost for window-0 DMAs
        cq="sync",            # DMA queue for one-time consts
        tailv=0,              # trailing windows with all-DVE chunks
        tail_dve=1,           # final windows: si-multiply on DVE (short chain)
        psl=3,                # PSUM bufs for the logit pool
        pspay=3,              # PSUM bufs for the payload-chunk pool
    )


# ===========================================================================
# Host-side packing
# ===========================================================================

def prep(nfeats, dst_feats, reward, src, dst, W_ns, b_ns, W_ni, W_nj, W_fij,
         attn, b_e, cfg=None):
    """Sort/shard/pack everything. Returns (meta, in_maps)."""
    cfg = cfg or default_cfg()
    n_dst = cfg["n_dst"]
    t_half = cfg["t_half"]
    span = cfg["span"]
    slots = 2 * t_half * 128    # slots per window
    t_w = 2 * t_half

    e_tot = src.shape[0]

    nfeats = np.asarray(nfeats, np.float32)
    dst_feats = np.asarray(dst_feats, np.float32)
    reward = np.asarray(reward, np.float32)
    src = np.asarray(src, np.int64)
    dst = np.asarray(dst, np.int64)
    W_ns = np.asarray(W_ns, np.float32)
    b_ns = np.asarray(b_ns, np.float32)
    W_ni = np.asarray(W_ni, np.float32)
    W_nj = np.asarray(W_nj, np.float32)
    W_fij = np.asarray(W_fij, np.float32)
    attn = np.asarray(attn, np.float32)
    b_e = np.asarray(b_e, np.float32)

    # ---- sort by dst and shard at dst boundaries --------------------------
    order = np.argsort(dst, kind="stable")
    d_s = dst[order]
    s_s = src[order]
    r_s = reward[order]

    cut = [0]
    for c in range(1, N_CORES):
        t = (e_tot * c) // N_CORES
        while t < e_tot and t > 0 and d_s[t] == d_s[t - 1]:
            t += 1
        cut.append(t)
    cut.append(e_tot)

    # ---- greedy window packing per core -----------------------------------
    per_core = []
    for c in range(N_CORES):
        e0, e1 = cut[c], cut[c + 1]
        d = d_s[e0:e1]
        wins = []  # (base, w_start, w_count) over local positions
        if e1 > e0:
            uniq, starts = np.unique(d, return_index=True)
            ends = np.append(starts[1:], len(d))
            base = None
            w_start = 0
            w_count = 0
            for gi in range(len(uniq)):
                dd = int(uniq[gi])
                glen = int(ends[gi] - starts[gi])
                if (base is None or dd - base > span - 1
                        or w_count + glen > slots):
                    if base is not None:
                        wins.append((base, w_start, w_count))
                    base = dd
                    w_start = int(starts[gi])
                    w_count = 0
                w_count += glen
            wins.append((base, w_start, w_count))
        per_core.append((e0, e1, wins))

    W = max(1, max(len(pc[2]) for pc in per_core))

    # virtual feature rows: x_row @ W_nj == colsum(W_fij); y_row @ W_nj == b_e
    wsum = W_fij.sum(axis=0)
    x_row = np.linalg.lstsq(W_nj.T.astype(np.float64), wsum.astype(np.float64),
                            rcond=None)[0].astype(np.float32)
    y_row = np.linalg.lstsq(W_nj.T.astype(np.float64), b_e.astype(np.float64),
                            rcond=None)[0].astype(np.float32)

    mf_all = []     # [128, W, t_w, 128] fp8 one-hot per slot
    zfe_all = []    # per core (zfe bf16 [128, W*slots], dfe8 [64, 2, W*slots])
    asm = []        # per core (slot_rows, global_rows)

    for c in range(N_CORES):
        e0, e1, wins = per_core[c]
        d = d_s[e0:e1]
        s = s_s[e0:e1]
        r = r_s[e0:e1]

        drel = np.full((W, slots), -1.0, np.float32)
        nfe = np.zeros((W * slots, IN_NODE), np.float32)
        dfe = np.zeros((W * slots, IN_NODE), np.float32)
        rows_slot = []
        rows_glob = []
        for w, (base, ws, wc) in enumerate(wins):
            sl = slice(ws, ws + wc)
            drel[w, :wc] = (d[sl] - base).astype(np.float32)
            nfe[w * slots:w * slots + wc] = nfeats[s[sl]]
            dfe[w * slots:w * slots + wc] = (dst_feats[d[sl]]
                                             + r[sl, None] * x_row[None, :]
                                             + y_row[None, :])
            uds = np.unique(d[sl])
            rows_slot.append(w * 128 + (uds - base))
            rows_glob.append(uds)

        # one-hot per slot, layout [128 p, W, t, 128 dcol]
        ohm = (drel.reshape(W, t_w, 128)[:, :, :, None]
               == np.arange(128, dtype=np.float32)).astype(FP8)
        ohm = np.ascontiguousarray(ohm.transpose(2, 0, 1, 3))

        zfe = np.ascontiguousarray(
            nfe.T.reshape(IN_NODE, W * slots).astype(BF16))
        if cfg.get("use_dr", True):
            # dfe channel-split for DoubleRow, window-pair packed:
            # [ceil(W/2), 128, 2, slots] -- even window on partitions 0:64,
            # odd window on 64:128, so each DMA moves a full-width tile.
            d4 = dfe.reshape(W, slots, 2, 64).transpose(0, 3, 2, 1).astype(FP8)
            if W % 2:
                d4 = np.concatenate(
                    [d4, np.zeros_like(d4[:1])], axis=0)
            df8 = np.ascontiguousarray(
                np.concatenate([d4[0::2], d4[1::2]], axis=1))
        else:
            df8 = np.ascontiguousarray(
                dfe.T.reshape(IN_NODE, W * slots).astype(FP8))
        mf_all.append(ohm)
        zfe_all.append((zfe, df8))
        asm.append((np.concatenate(rows_slot) if rows_slot else
                    np.zeros(0, np.int64),
                    np.concatenate(rows_glob) if rows_glob else
                    np.zeros(0, np.int64)))

    # ---- shared constants -------------------------------------------------
    # e = sum_f attn*leaky(x) ~= 0.99*sum_f attn*relu(x)  (relu
    # decomposition of leaky_relu; the 0.99 is folded into attn_rep and
    # the 0.01-slope lin term is dropped -- its logit contribution sits
    # below bf16 rounding of eat, verified vs the reference).
    wcat = np.concatenate([W_ni * LSCALE, W_ns * 0.25],
                          axis=1).astype(BF16)                       # [128,320]
    wnj_full = W_nj * LSCALE
    if cfg.get("use_dr", True):
        w2 = wnj_full.reshape(2, 64, FE).transpose(1, 0, 2).astype(FP8)
        wnj2 = np.ascontiguousarray(np.concatenate([w2, w2], axis=0))
    else:
        wnj2 = np.ascontiguousarray(wnj_full.astype(FP8))
    attn_rep = np.broadcast_to(
        (0.99 / LSCALE) * attn.reshape(-1).astype(np.float32),
        (128, H * OUT_EDGE)).astype(BF16).copy()
    bmean = np.broadcast_to(b_ns.reshape(H, OUT_NODE).mean(axis=0),
                            (128, OUT_NODE)).astype(BF16).copy()
    negb = (-bmean.astype(np.float32)).astype(BF16)

    consts = np.concatenate(
        [wcat, attn_rep, bmean, negb], axis=1)                   # [128, 512]
    in_maps = []
    for c in range(N_CORES):
        in_maps.append(dict(
            zfe=zfe_all[c][0], dfe8=zfe_all[c][1], ohm=mf_all[c],
            consts=consts, wnj2=wnj2,
        ))

    meta = dict(W=W, asm=asm, cfg=cfg)
    return meta, in_maps


# ===========================================================================
# Device program
# ===========================================================================

def build_program(W, cfg, fused_chunks=None):
    import concourse.bacc as bacc
    import concourse.tile as tile
    import concourse.mybir as mybir
    from contextlib import ExitStack

    dt = mybir.dt
    AF = mybir.ActivationFunctionType
    OP = mybir.AluOpType
    PM = mybir.MatmulPerfMode

    if fused_chunks is None:
        fused_chunks = tuple(cfg.get("fused", (1,)))
    t_half = cfg["t_half"]
    t_w = 2 * t_half
    slots = t_w * 128
    TCH = cfg.get("tch", 4)      # tiles per payload psum chunk
    n_ch = t_w // TCH            # payload chunks per window
    OB = cfg.get("ob", 4)       # windows per output DMA batch

    nc = bacc.Bacc(None, target_bir_lowering=False)

    ZFE = nc.declare_dram_parameter("zfe", [IN_NODE, W * slots],
                                    dt.bfloat16, isOutput=False)
    use_dr = cfg.get("use_dr", True)
    if use_dr:
        DFE8 = nc.declare_dram_parameter("dfe8",
                                         [(W + 1) // 2, 128, 2, slots],
                                         dt.float8e4, isOutput=False)
        WNJ2 = nc.declare_dram_parameter("wnj2", [128, 2, FE], dt.float8e4,
                                         isOutput=False)
    else:
        DFE8 = nc.declare_dram_parameter("dfe8", [IN_NODE, W * slots],
                                         dt.float8e4, isOutput=False)
        WNJ2 = nc.declare_dram_parameter("wnj2", [IN_NODE, FE],
                                         dt.float8e4, isOutput=False)
    OHM = nc.declare_dram_parameter("ohm", [128, W, t_w, 128],
                                    dt.float8e4, isOutput=False)
    CONSTS = nc.declare_dram_parameter("consts", [IN_NODE, NPROJ + 3 * FE],
                                       dt.bfloat16, isOutput=False)
    OUT = nc.declare_dram_parameter("out", [128, W, OUT_NODE], dt.bfloat16,
                                    isOutput=True)

    with tile.TileContext(nc) as tc, ExitStack() as ctx:
        cpool = ctx.enter_context(tc.tile_pool(name="consts", bufs=1))
        call_s = cpool.tile([128, NPROJ + 3 * FE], dt.bfloat16)
        getattr(nc, cfg.get("cq", "scalar")).dma_start(call_s[:], CONSTS[:])
        wcat_s = call_s[:, 0:NPROJ]
        attn_s = call_s[:, NPROJ:NPROJ + FE]
        bmean_s = call_s[:, NPROJ + FE:NPROJ + FE + OUT_NODE]
        negb_s = call_s[:, NPROJ + FE + OUT_NODE:NPROJ + FE + 2 * OUT_NODE]
        if use_dr:
            wnj2_s = cpool.tile([128, 2, FE], dt.float8e4)
            getattr(nc, cfg.get("cq", "scalar")).dma_start(wnj2_s[:], WNJ2[:])
        else:
            wnjf_s = cpool.tile([128, FE], dt.float8e4)
            nc.sync.dma_start(wnjf_s[:], WNJ2[:])

        BUFS = cfg.get("bufs", (5, 5, 5, 8, 4, 6))
        with tc.tile_pool(name="feat", bufs=BUFS[0]) as fpool, \
             tc.tile_pool(name="meta", bufs=BUFS[1]) as mpool, \
             tc.tile_pool(name="lrp", bufs=BUFS[2]) as lpool, \
             tc.tile_pool(name="stgp", bufs=BUFS[3]) as spool, \
             tc.tile_pool(name="rhsp", bufs=BUFS[4]) as rpool, \
             tc.tile_pool(name="ep", bufs=BUFS[5]) as epool, \
             tc.tile_pool(name="psL", bufs=cfg.get("psl", 2),
                          space="PSUM") as psL, \
             tc.tile_pool(name="psPay", bufs=cfg.get("pspay", 8 // TCH),
                          space="PSUM") as psPay, \
             tc.tile_pool(name="psP", bufs=cfg.get("psp", 2),
                          space="PSUM") as psP:
            for w in range(W):
                import contextlib
                prio_ctx = (tc.high_priority()
                            if w == 0 and cfg.get("w0_prio", True)
                            else contextlib.nullcontext())
                with prio_ctx:
                    nfe = fpool.tile([128, slots], dt.bfloat16, tag="nfe")
                    nq = (nc.scalar if (w == 0 and cfg.get("w0_nfe_sc", False))
                          else nc.sync)
                    if w == 0 and cfg.get("w0_split", False):
                        hs = slots // 2
                        nq.dma_start(nfe[:, 0:hs], ZFE[:, 0:hs])
                        nq.dma_start(nfe[:, hs:slots], ZFE[:, hs:slots])
                    else:
                        nq.dma_start(nfe[:],
                                     ZFE[:, w * slots:(w + 1) * slots])
                if use_dr:
                    if w % 2 == 0:
                        dq = (nc.scalar if (w == 0 and cfg.get("w0_dfe_sc",
                                                               False))
                              else nc.sync)
                        with (tc.high_priority()
                              if w == 0 and cfg.get("w0_prio", True)
                              else contextlib.nullcontext()):
                            dfep = fpool.tile([128, 2, slots], dt.float8e4,
                                              tag="dfe", name="dfep")
                            if w == 0 and cfg.get("w0_split", False):
                                hs = slots // 2
                                dq.dma_start(dfep[:, :, 0:hs],
                                             DFE8[0, :, :, 0:hs])
                                dq.dma_start(dfep[:, :, hs:slots],
                                             DFE8[0, :, :, hs:slots])
                            else:
                                dq.dma_start(dfep[:], DFE8[w // 2, :, :, :])
                    p0 = 64 * (w % 2)
                    dfe = dfep[p0:p0 + 64, :, :]
                    wnjv = wnj2_s[p0:p0 + 64, :, :]
                else:
                    dfe = fpool.tile([128, slots], dt.float8e4, tag="dfe")
                    nc.sync.dma_start(dfe[:],
                                      DFE8[:, w * slots:(w + 1) * slots])
                ohw = mpool.tile([128, t_w, 128], dt.float8e4, tag="ohw")
                getattr(nc, cfg.get("ohw_q", "sync")).dma_start(
                    ohw[:], OHM[:, w, :, :])

                P = psP.tile([128, PW], dt.float32, tag="P")
                rhp = rpool.tile([128, t_w, PW], dt.bfloat16, tag="rhp")
                eat = lpool.tile([128, t_w, H], dt.bfloat16, tag="eat")
                wes = lpool.tile([128, t_w, H], dt.bfloat16, tag="wes")

                prPs = {}
                for hf in range(2):
                    ts0 = hf * t_half

                    def emit_pay(ts0=ts0, hf=hf):
                        for ci in range(t_half // TCH):
                            ch = hf * (t_half // TCH) + ci
                            prP = psPay.tile([128, TCH, NPAY], dt.float32,
                                             tag="prP", name="prP")
                            for k in range(TCH):
                                tl = ci * TCH + k
                                c0 = (ts0 + tl) * 128
                                nc.tensor.matmul(
                                    prP[:, k, :], lhsT=nfe[:, c0:c0 + 128],
                                    rhs=call_s[:, FE:NPROJ], start=True,
                                    stop=True, skip_group_check=True)
                            prPs[ch] = prP

                    if w == 0 and cfg.get("w0_payfirst", False):
                        emit_pay()
                    prL = psL.tile([128, t_half, FE], dt.float32, tag="prL")
                    for tl in range(t_half):
                        t = ts0 + tl
                        c0 = t * 128
                        nc.tensor.matmul(
                            prL[:, tl, :], lhsT=nfe[:, c0:c0 + 128],
                            rhs=call_s[:, 0:FE], start=True, stop=False,
                            skip_group_check=True)
                        if use_dr:
                            nc.tensor.matmul(
                                prL[:, tl, :], lhsT=dfe[:, :, c0:c0 + 128],
                                rhs=wnjv, start=False, stop=True,
                                perf_mode=PM.DoubleRow, skip_group_check=True)
                        else:
                            nc.tensor.matmul(
                                prL[:, tl, :], lhsT=dfe[:, c0:c0 + 128],
                                rhs=wnjf_s[:, 0:FE], start=False, stop=True,
                                skip_group_check=True)
                    # fused relu+attn on DVE (reads PSUM), then head reduce
                    ea = lpool.tile([128, t_half, FE], dt.bfloat16, tag="ea")
                    import contextlib as _ctl
                    with (tc.high_priority(offset=cfg.get("stt_prio", 0))
                          if cfg.get("stt_prio", 0) else _ctl.nullcontext()):
                        nc.vector.scalar_tensor_tensor(
                            out=ea[:], in0=prL[:], scalar=0.0,
                            in1=attn_s.unsqueeze(1).broadcast_to(
                                [128, t_half, FE]),
                            op0=OP.max, op1=OP.mult)
                    # attn-dot reduce of 16 as a 4-level add tree:
                    # lvl1-2 on Pool, lvl3-4 on DVE (2x-capable widths)
                    eav = ea[:].rearrange("p t (h f) -> p t h f", f=OUT_EDGE)
                    tr1 = lpool.tile([128, t_half, H, 8], dt.bfloat16,
                                     tag="tr1")
                    nc.gpsimd.tensor_tensor(out=tr1[:], in0=eav[:, :, :, 0:8],
                                            in1=eav[:, :, :, 8:16], op=OP.add)
                    tr2 = lpool.tile([128, t_half, H, 4], dt.bfloat16,
                                     tag="tr2")
                    nc.gpsimd.tensor_tensor(out=tr2[:], in0=tr1[:, :, :, 0:4],
                                            in1=tr1[:, :, :, 4:8], op=OP.add)
                    tr3 = lpool.tile([128, t_half, H, 2], dt.bfloat16,
                                     tag="tr3")
                    e34 = getattr(nc, cfg.get("tree34", "gpsimd"))
                    e34.tensor_tensor(out=tr3[:], in0=tr2[:, :, :, 0:2],
                                      in1=tr2[:, :, :, 2:4], op=OP.add)
                    e34.tensor_tensor(
                        out=eat[:, ts0:ts0 + t_half, :],
                        in0=tr3[:, :, :, 0], in1=tr3[:, :, :, 1], op=OP.add)
                    if cfg.get("exp_half", True) or (
                            w >= W - cfg.get("tail_eh", 0)):
                        nc.scalar.activation(wes[:, ts0:ts0 + t_half, :],
                                             eat[:, ts0:ts0 + t_half, :],
                                             AF.Exp)
                    if not (w == 0 and cfg.get("w0_payfirst", False)):
                        emit_pay()

                # ---- weights: we = exp(eat) -------------------------------
                if not (cfg.get("exp_half", True)
                        or w >= W - cfg.get("tail_eh", 0)):
                    nc.scalar.activation(wes[:], eat[:], AF.Exp)

                # ---- payload x weight per chunk ---------------------------
                nc.gpsimd.tensor_scalar(
                    out=rhp[:, :, NPAY:PW], in0=wes[:],
                    scalar1=1.0, scalar2=None, op0=OP.mult)
                chasg = cfg.get("chasg", None)
                if chasg and w >= W - cfg.get("tailv", 0):
                    chasg = "v" * n_ch
                for ch in range(n_ch):
                    t0 = ch * TCH
                    w_rep = (wes[:, t0:t0 + TCH, :].unsqueeze(3)
                             .broadcast_to([128, TCH, H, OUT_NODE]))
                    out_v = (rhp[:, t0:t0 + TCH, 0:NPAY]
                             .rearrange("p t (h f) -> p t h f", f=OUT_NODE))
                    kind = (chasg[ch] if chasg
                            else ("v" if ch in fused_chunks else "a"))
                    if kind == "v":
                        # fused crossing+mult from psum on DVE
                        nc.vector.tensor_tensor(
                            out=out_v,
                            in0=prPs[ch][:, :, :].rearrange(
                                "p t (h f) -> p t h f", f=OUT_NODE),
                            in1=w_rep, op=OP.mult)
                    else:
                        stg = spool.tile([128, TCH, NPAY], dt.bfloat16,
                                         tag="stg")
                        nc.scalar.copy(stg[:], prPs[ch][:, :, :])
                        eng = nc.gpsimd if kind == "a" else nc.vector
                        eng.tensor_tensor(
                            out=out_v,
                            in0=stg[:].rearrange("p t (h f) -> p t h f",
                                                 f=OUT_NODE),
                            in1=w_rep, op=OP.mult)

                # ---- scatter ---------------------------------------------
                for t in range(t_w):
                    nc.tensor.matmul(P[:], lhsT=ohw[:, t, :],
                                     rhs=rhp[:, t, :],
                                     start=(t == 0), stop=(t == t_w - 1),
                                     skip_group_check=True)

                # ---- epilogue --------------------------------------------
                pb = epool.tile([128, PW], dt.bfloat16, tag="pb")
                with tc.high_priority(offset=cfg.get("prio", 300)):
                    if cfg.get("pb_eng", "vector") == "vector":
                        nc.vector.tensor_scalar(
                            out=pb[:], in0=P[:], scalar1=0.0, scalar2=None,
                            op0=OP.add)
                    else:
                        nc.scalar.copy(pb[:], P[:])
                sg = epool.tile([128, H, 1], dt.float32, tag="sg")
                with tc.high_priority(offset=cfg.get("prio", 300)):
                    nc.vector.tensor_scalar(
                        out=sg[:],
                        in0=P[:, NPAY:PW].rearrange("p (h b) -> p h b", b=1),
                        scalar1=1e-30, scalar2=None, op0=OP.max)
                si = epool.tile([128, H, 1], dt.float32, tag="si")
                nc.vector.reciprocal(si[:], sg[:])
                tmp = epool.tile([128, H, OUT_NODE], dt.bfloat16, tag="tmp")
                tmp_eng = (nc.vector
                           if w >= W - cfg.get("tail_dve", 0) else nc.gpsimd)
                tmp_eng.tensor_tensor(
                    out=tmp[:],
                    in0=pb[:, 0:NPAY].rearrange("p (h f) -> p h f",
                                                f=OUT_NODE),
                    in1=si[:].broadcast_to([128, H, OUT_NODE]),
                    op=OP.mult)
                ob = w % OB
                if ob == 0:
                    outf = epool.tile([128, OB, OUT_NODE], dt.bfloat16,
                                      tag="outf", name="outf")
                    outf_cur = outf
                with nc.allow_low_precision("4-term head mean; output is "
                                            "bf16 anyway"):
                    nc.vector.tensor_reduce(
                        outf_cur[:, ob, :].unsqueeze(2),
                        tmp[:].rearrange("p h f -> p f h"),
                        axis=mybir.AxisListType.X, op=OP.add)
                if ob == OB - 1 or w == W - 1:
                    # bias + relu:  relu(x + b) == max(x, -b) + b
                    nb = ob + 1
                    nc.vector.tensor_tensor(
                        out=outf_cur[:, 0:nb, :], in0=outf_cur[:, 0:nb, :],
                        in1=negb_s.unsqueeze(1).broadcast_to(
                            [128, nb, OUT_NODE]), op=OP.max)
                    nc.vector.tensor_tensor(
                        out=outf_cur[:, 0:nb, :], in0=outf_cur[:, 0:nb, :],
                        in1=bmean_s.unsqueeze(1).broadcast_to(
                            [128, nb, OUT_NODE]), op=OP.add)
                    w0 = w - ob
                    getattr(nc, cfg.get("out_q", "scalar")).dma_start(
                        OUT[:, w0:w + 1, :], outf_cur[:, 0:nb, :])

    if not nc.is_finalized():
        nc.finalize()
    return nc


# ===========================================================================
# numpy emulation of the device program (for validation/debug)
# ===========================================================================

def emulate_core(in_map, W, cfg):
    t_half = cfg["t_half"]
    slots = 2 * t_half * 128

    f32 = np.float32
    consts = in_map["consts"].astype(f32)
    wcat = consts[:, 0:NPROJ]
    wnj2 = in_map["wnj2"].astype(f32)
    if wnj2.ndim == 3:
        wnj = wnj2.transpose(1, 0, 2).reshape(IN_NODE, FE)
    else:
        wnj = wnj2
    attn_rep = consts[0, NPROJ:NPROJ + FE]
    bmean = consts[0, NPROJ + FE:NPROJ + FE + OUT_NODE]

    out = np.zeros((W * 128, OUT_NODE), f32)
    for w in range(W):
        nfe = in_map["zfe"][:, w * slots:(w + 1) * slots].astype(f32).T
        d8 = in_map["dfe8"]
        if d8.ndim == 4:
            p0 = 64 * (w % 2)
            dfe = (d8[w // 2, p0:p0 + 64].astype(f32).transpose(1, 0, 2)
                   .reshape(IN_NODE, slots).T)
        else:
            dfe = d8[:, w * slots:(w + 1) * slots].astype(f32).T
        pay = (nfe @ wcat[:, FE:NPROJ]).astype(BF16).astype(f32)
        fout = (nfe @ wcat[:, 0:FE] + dfe @ wnj[:, 0:FE])
        r = (np.maximum(fout, 0.0) * attn_rep[None, :]).astype(BF16).astype(f32)
        eat = (r.reshape(-1, H, OUT_EDGE).sum(axis=2)).astype(BF16).astype(f32)
        wgt = np.exp(eat).astype(BF16).astype(f32)            # [slots, H]
        oh = (in_map["ohm"][:, w].astype(f32).transpose(1, 0, 2)
              .reshape(slots, 128))
        rhs = np.concatenate(
            [(pay.reshape(-1, H, OUT_NODE)
              * wgt[:, :, None]).reshape(-1, NPAY).astype(BF16).astype(f32),
             wgt], axis=1)
        P = oh.T @ rhs                                        # [128, 260]
        pb = P.astype(BF16).astype(f32)
        s = np.maximum(P[:, NPAY:PW], 1e-30)
        tmp = (pb[:, 0:NPAY].reshape(128, H, OUT_NODE)
               / s[:, :, None]).astype(BF16).astype(f32)
        acc = tmp.sum(axis=1).astype(BF16).astype(f32)
        res = np.maximum(acc, -bmean[None, :]) + bmean[None, :]
        out[w * 128:(w + 1) * 128] = res.astype(BF16).astype(f32)
    return out


def assemble(meta, results):
    n_dst = meta["cfg"]["n_dst"]
    out = np.zeros((n_dst, OUT_NODE), np.float32)
    for c in range(N_CORES):
        slots_rows, glob_rows = meta["asm"][c]
        if len(glob_rows):
            flat = (results[c]["out"].astype(np.float32)
                    .transpose(1, 0, 2).reshape(-1, OUT_NODE))
            out[glob_rows] = flat[slots_rows]
    return out


# ===========================================================================
# entry point
# ===========================================================================

_CACHE = {}
LAST_EXEC_NS = None
LAST_RESULT = None


def kernel(nfeats, dst_feats, reward, src, dst,
           W_ns, b_ns, W_ni, W_nj, W_fij, attn, b_e):
    global LAST_EXEC_NS, LAST_RESULT
    import os
    from concourse.bass_utils import run_bass_kernel_spmd

    meta, in_maps = prep(nfeats, dst_feats, reward, src, dst,
                         W_ns, b_ns, W_ni, W_nj, W_fij, attn, b_e)
    key = meta["W"]
    if key not in _CACHE:
        _CACHE[key] = build_program(meta["W"], meta["cfg"])
    nc = _CACHE[key]
    kwargs = {}
    if os.environ.get("EGAT_TRACE"):
        kwargs = dict(trace=True)
    try:
        res = run_bass_kernel_spmd(nc, in_maps, list(range(N_CORES)), **kwargs)
    except ModuleNotFoundError:
        res = run_bass_kernel_spmd(nc, in_maps, list(range(N_CORES)))
    LAST_EXEC_NS = res.exec_time_ns
    LAST_RESULT = res
    return assemble(meta, res.results)


def estimate_ns(W=None, cfg=None):
    """Cost-model (no_exec CoreSim) estimate of the per-core kernel time.

    Always builds a fresh program: sharing an nc between CoreSim and a
    real run corrupts both (the run inflates the estimate, and a prior
    estimate breaks the subsequent compile).
    """
    from concourse.bass_interp import CoreSim
    cfg = cfg or default_cfg()
    if W is None:
        W = sorted(_CACHE)[0] if _CACHE else 50
    nc = build_program(W, cfg)
    sim = CoreSim(nc, no_exec=True, publish_trace=False)
    sim.simulate()
    return int(sim.time)


# revision 43
# speedup vs baseline: 1.0006x; 1.0005x over previous
"""EGAT kernel v2 for 8 Trainium2 NeuronCores.

Edge-parallel windowed design: edges sorted by dst, 8 dst-disjoint shards,
~50 windows/core of 2048 edge slots spanning <=128 dst rows.  Per window:
project gathered src/dst features to logits and payload, softmax weights
via exp of the attn dot, weight the payload, and aggregate per dst row
with a one-hot scatter matmul.  v2 rebalance vs the v1.5 baseline:

- dst-side logit projection is one fp8 DoubleRow matmul per tile (split-K
  over channel halves): 32 cyc/tile vs 64.  dfe tiles of consecutive
  window pairs share one full-width DMA ([ceil(W/2), 128, 2, slots];
  even window on partitions 0:64, odd on 64:128, wnj2 duplicated per
  half) -- the DMA cost model charges per-partition bytes, so a
  64-partition transfer would pay double.
- relu+attn fused into one DVE scalar_tensor_tensor per half window
  (replaces the ACT relu cross + Pool attn multiply).
- the 0.01-slope lin term of leaky_relu is dropped: its logit
  contribution sits below bf16 rounding of eat (verified vs reference).
- attn-dot reduce of 16 runs as a 4-level add tree (lvl1-2 Pool,
  lvl3-4 Pool) instead of one DVE tensor_reduce.
- one merged exp per window; its [128,t,4] bf16 output feeds the scatter
  weight columns (copied into rhp) and the payload multiplies directly.
- payload PSUM crossing in 8 chunks of 2 tiles: per-chunk engine
  pattern "aavaavva" (a = ACT-stage + Pool-mult, v = DVE fused mult
  from PSUM).
- scatter is a single 260-col matmul per tile (payload + weight sums).
- epilogue: ACT crosses P once; sg/recip on DVE in f32; si-multiply on
  Pool; head-mean reduce + batched bias/relu on DVE; output DMA per 3
  windows into a [128, W, 64] layout.
- bf16 consts ride one [128, 512] DMA; all DMAs on the SP queue.

PSUM banks: logit pool 3 bufs, payload-chunk pool 3, scatter pool 2
(the third logit buffer relieves the prL-recycle stall on the PE).

Cost-model estimate 218174 ns/core (baseline 282281); HW-verified
rel err 0.0051 (gate 2e-2).
"""

import sys

sys.path.insert(0, "/opt/trn_rl_repo")

import numpy as np
import ml_dtypes

BF16 = ml_dtypes.bfloat16
FP8 = ml_dtypes.float8_e4m3

# ---- problem constants (hardcoded per the task contract) -------------------
N_SRC = 50000
N_DST = 50000
E = 800000
IN_NODE = 128
IN_EDGE = 16
OUT_NODE = 64
OUT_EDGE = 16
H = 4
SLOPE = 0.01

N_CORES = 8

FE = H * OUT_EDGE            # 64 logit cols
NPAY = H * OUT_NODE          # 256 payload cols
NPROJ = FE + NPAY            # 320
PW = NPAY + H                # 260 scatter cols (payload + 4 weight sums)
LSCALE = 8.0                 # logit-projection scale (fp8 subnormal dodge)


def default_cfg():
    return dict(
        n_dst=N_DST,
        t_half=8,             # tiles per half-window (8 -> 1024 slots/half)
        span=128,             # max dst rows per window
        use_dr=True,          # fp8 DoubleRow for dst-side logit matmuls
        ohw_q="sync",         # DMA queue for the one-hot
        pb_eng="scalar",      # engine for the P psum->sbuf cross
        tree34="gpsimd",      # engine for eat-tree levels 3-4
        prio=300,             # priority boost for P_ext-releasing ops
        tch=2,                # tiles per payload psum chunk
        chasg="aavaavva",     # per-chunk engine: a=ACT+Pool, v=DVE-fused
        out_q="sync",         # DMA queue for the output
        ob=3,                 # windows per output DMA batch
        exp_half=False,       # one merged exp per window
        w0_prio=False,        # priority boost for window-0 DMAs
        cq="sync",            # DMA queue for one-time consts
        tailv=0,              # trailing windows with all-DVE chunks
        tail_dve=1,           # final windows: si-multiply on DVE (short chain)
        tail_pb=1,            # final windows: P-cross on DVE (short chain)
        psl=3,                # PSUM bufs for the logit pool
        pspay=3,              # PSUM bufs for the payload-chunk pool
    )


# ===========================================================================
# Host-side packing
# ===========================================================================

def prep(nfeats, dst_feats, reward, src, dst, W_ns, b_ns, W_ni, W_nj, W_fij,
         attn, b_e, cfg=None):
    """Sort/shard/pack everything. Returns (meta, in_maps)."""
    cfg = cfg or default_cfg()
    n_dst = cfg["n_dst"]
    t_half = cfg["t_half"]
    span = cfg["span"]
    slots = 2 * t_half * 128    # slots per window
    t_w = 2 * t_half

    e_tot = src.shape[0]

    nfeats = np.asarray(nfeats, np.float32)
    dst_feats = np.asarray(dst_feats, np.float32)
    reward = np.asarray(reward, np.float32)
    src = np.asarray(src, np.int64)
    dst = np.asarray(dst, np.int64)
    W_ns = np.asarray(W_ns, np.float32)
    b_ns = np.asarray(b_ns, np.float32)
    W_ni = np.asarray(W_ni, np.float32)
    W_nj = np.asarray(W_nj, np.float32)
    W_fij = np.asarray(W_fij, np.float32)
    attn = np.asarray(attn, np.float32)
    b_e = np.asarray(b_e, np.float32)

    # ---- sort by dst and shard at dst boundaries --------------------------
    order = np.argsort(dst, kind="stable")
    d_s = dst[order]
    s_s = src[order]
    r_s = reward[order]

    cut = [0]
    for c in range(1, N_CORES):
        t = (e_tot * c) // N_CORES
        while t < e_tot and t > 0 and d_s[t] == d_s[t - 1]:
            t += 1
        cut.append(t)
    cut.append(e_tot)

    # ---- greedy window packing per core -----------------------------------
    per_core = []
    for c in range(N_CORES):
        e0, e1 = cut[c], cut[c + 1]
        d = d_s[e0:e1]
        wins = []  # (base, w_start, w_count) over local positions
        if e1 > e0:
            uniq, starts = np.unique(d, return_index=True)
            ends = np.append(starts[1:], len(d))
            base = None
            w_start = 0
            w_count = 0
            for gi in range(len(uniq)):
                dd = int(uniq[gi])
                glen = int(ends[gi] - starts[gi])
                if (base is None or dd - base > span - 1
                        or w_count + glen > slots):
                    if base is not None:
                        wins.append((base, w_start, w_count))
                    base = dd
                    w_start = int(starts[gi])
                    w_count = 0
                w_count += glen
            wins.append((base, w_start, w_count))
        per_core.append((e0, e1, wins))

    W = max(1, max(len(pc[2]) for pc in per_core))

    # virtual feature rows: x_row @ W_nj == colsum(W_fij); y_row @ W_nj == b_e
    wsum = W_fij.sum(axis=0)
    x_row = np.linalg.lstsq(W_nj.T.astype(np.float64), wsum.astype(np.float64),
                            rcond=None)[0].astype(np.float32)
    y_row = np.linalg.lstsq(W_nj.T.astype(np.float64), b_e.astype(np.float64),
                            rcond=None)[0].astype(np.float32)

    mf_all = []     # [128, W, t_w, 128] fp8 one-hot per slot
    zfe_all = []    # per core (zfe bf16 [128, W*slots], dfe8 [64, 2, W*slots])
    asm = []        # per core (slot_rows, global_rows)

    for c in range(N_CORES):
        e0, e1, wins = per_core[c]
        d = d_s[e0:e1]
        s = s_s[e0:e1]
        r = r_s[e0:e1]

        drel = np.full((W, slots), -1.0, np.float32)
        nfe = np.zeros((W * slots, IN_NODE), np.float32)
        dfe = np.zeros((W * slots, IN_NODE), np.float32)
        rows_slot = []
        rows_glob = []
        for w, (base, ws, wc) in enumerate(wins):
            sl = slice(ws, ws + wc)
            drel[w, :wc] = (d[sl] - base).astype(np.float32)
            nfe[w * slots:w * slots + wc] = nfeats[s[sl]]
            dfe[w * slots:w * slots + wc] = (dst_feats[d[sl]]
                                             + r[sl, None] * x_row[None, :]
                                             + y_row[None, :])
            uds = np.unique(d[sl])
            rows_slot.append(w * 128 + (uds - base))
            rows_glob.append(uds)

        # one-hot per slot, layout [128 p, W, t, 128 dcol]
        ohm = (drel.reshape(W, t_w, 128)[:, :, :, None]
               == np.arange(128, dtype=np.float32)).astype(FP8)
        ohm = np.ascontiguousarray(ohm.transpose(2, 0, 1, 3))

        zfe = np.ascontiguousarray(
            nfe.T.reshape(IN_NODE, W * slots).astype(BF16))
        if cfg.get("use_dr", True):
            # dfe channel-split for DoubleRow, window-pair packed:
            # [ceil(W/2), 128, 2, slots] -- even window on partitions 0:64,
            # odd window on 64:128, so each DMA moves a full-width tile.
            d4 = dfe.reshape(W, slots, 2, 64).transpose(0, 3, 2, 1).astype(FP8)
            if W % 2:
                d4 = np.concatenate(
                    [d4, np.zeros_like(d4[:1])], axis=0)
            df8 = np.ascontiguousarray(
                np.concatenate([d4[0::2], d4[1::2]], axis=1))
        else:
            df8 = np.ascontiguousarray(
                dfe.T.reshape(IN_NODE, W * slots).astype(FP8))
        mf_all.append(ohm)
        zfe_all.append((zfe, df8))
        asm.append((np.concatenate(rows_slot) if rows_slot else
                    np.zeros(0, np.int64),
                    np.concatenate(rows_glob) if rows_glob else
                    np.zeros(0, np.int64)))

    # ---- shared constants -------------------------------------------------
    # e = sum_f attn*leaky(x) ~= 0.99*sum_f attn*relu(x)  (relu
    # decomposition of leaky_relu; the 0.99 is folded into attn_rep and
    # the 0.01-slope lin term is dropped -- its logit contribution sits
    # below bf16 rounding of eat, verified vs the reference).
    wcat = np.concatenate([W_ni * LSCALE, W_ns * 0.25],
                          axis=1).astype(BF16)                       # [128,320]
    wnj_full = W_nj * LSCALE
    if cfg.get("use_dr", True):
        w2 = wnj_full.reshape(2, 64, FE).transpose(1, 0, 2).astype(FP8)
        wnj2 = np.ascontiguousarray(np.concatenate([w2, w2], axis=0))
    else:
        wnj2 = np.ascontiguousarray(wnj_full.astype(FP8))
    attn_rep = np.broadcast_to(
        (0.99 / LSCALE) * attn.reshape(-1).astype(np.float32),
        (128, H * OUT_EDGE)).astype(BF16).copy()
    bmean = np.broadcast_to(b_ns.reshape(H, OUT_NODE).mean(axis=0),
                            (128, OUT_NODE)).astype(BF16).copy()
    negb = (-bmean.astype(np.float32)).astype(BF16)

    consts = np.concatenate(
        [wcat, attn_rep, bmean, negb], axis=1)                   # [128, 512]
    in_maps = []
    for c in range(N_CORES):
        in_maps.append(dict(
            zfe=zfe_all[c][0], dfe8=zfe_all[c][1], ohm=mf_all[c],
            consts=consts, wnj2=wnj2,
        ))

    meta = dict(W=W, asm=asm, cfg=cfg)
    return meta, in_maps


# ===========================================================================
# Device program
# ===========================================================================

def build_program(W, cfg, fused_chunks=None):
    import concourse.bacc as bacc
    import concourse.tile as tile
    import concourse.mybir as mybir
    from contextlib import ExitStack

    dt = mybir.dt
    AF = mybir.ActivationFunctionType
    OP = mybir.AluOpType
    PM = mybir.MatmulPerfMode

    if fused_chunks is None:
        fused_chunks = tuple(cfg.get("fused", (1,)))
    t_half = cfg["t_half"]
    t_w = 2 * t_half
    slots = t_w * 128
    TCH = cfg.get("tch", 4)      # tiles per payload psum chunk
    n_ch = t_w // TCH            # payload chunks per window
    OB = cfg.get("ob", 4)       # windows per output DMA batch

    nc = bacc.Bacc(None, target_bir_lowering=False)

    ZFE = nc.declare_dram_parameter("zfe", [IN_NODE, W * slots],
                                    dt.bfloat16, isOutput=False)
    use_dr = cfg.get("use_dr", True)
    if use_dr:
        DFE8 = nc.declare_dram_parameter("dfe8",
                                         [(W + 1) // 2, 128, 2, slots],
                                         dt.float8e4, isOutput=False)
        WNJ2 = nc.declare_dram_parameter("wnj2", [128, 2, FE], dt.float8e4,
                                         isOutput=False)
    else:
        DFE8 = nc.declare_dram_parameter("dfe8", [IN_NODE, W * slots],
                                         dt.float8e4, isOutput=False)
        WNJ2 = nc.declare_dram_parameter("wnj2", [IN_NODE, FE],
                                         dt.float8e4, isOutput=False)
    OHM = nc.declare_dram_parameter("ohm", [128, W, t_w, 128],
                                    dt.float8e4, isOutput=False)
    CONSTS = nc.declare_dram_parameter("consts", [IN_NODE, NPROJ + 3 * FE],
                                       dt.bfloat16, isOutput=False)
    OUT = nc.declare_dram_parameter("out", [128, W, OUT_NODE], dt.bfloat16,
                                    isOutput=True)

    with tile.TileContext(nc) as tc, ExitStack() as ctx:
        cpool = ctx.enter_context(tc.tile_pool(name="consts", bufs=1))
        call_s = cpool.tile([128, NPROJ + 3 * FE], dt.bfloat16)
        getattr(nc, cfg.get("cq", "scalar")).dma_start(call_s[:], CONSTS[:])
        wcat_s = call_s[:, 0:NPROJ]
        attn_s = call_s[:, NPROJ:NPROJ + FE]
        bmean_s = call_s[:, NPROJ + FE:NPROJ + FE + OUT_NODE]
        negb_s = call_s[:, NPROJ + FE + OUT_NODE:NPROJ + FE + 2 * OUT_NODE]
        if use_dr:
            wnj2_s = cpool.tile([128, 2, FE], dt.float8e4)
            getattr(nc, cfg.get("cq", "scalar")).dma_start(wnj2_s[:], WNJ2[:])
        else:
            wnjf_s = cpool.tile([128, FE], dt.float8e4)
            nc.sync.dma_start(wnjf_s[:], WNJ2[:])

        BUFS = cfg.get("bufs", (5, 5, 5, 8, 4, 6))
        with tc.tile_pool(name="feat", bufs=BUFS[0]) as fpool, \
             tc.tile_pool(name="meta", bufs=BUFS[1]) as mpool, \
             tc.tile_pool(name="lrp", bufs=BUFS[2]) as lpool, \
             tc.tile_pool(name="stgp", bufs=BUFS[3]) as spool, \
             tc.tile_pool(name="rhsp", bufs=BUFS[4]) as rpool, \
             tc.tile_pool(name="ep", bufs=BUFS[5]) as epool, \
             tc.tile_pool(name="psL", bufs=cfg.get("psl", 2),
                          space="PSUM") as psL, \
             tc.tile_pool(name="psPay", bufs=cfg.get("pspay", 8 // TCH),
                          space="PSUM") as psPay, \
             tc.tile_pool(name="psP", bufs=cfg.get("psp", 2),
                          space="PSUM") as psP:
            for w in range(W):
                import contextlib
                prio_ctx = (tc.high_priority()
                            if w == 0 and cfg.get("w0_prio", True)
                            else contextlib.nullcontext())
                with prio_ctx:
                    nfe = fpool.tile([128, slots], dt.bfloat16, tag="nfe")
                    nq = (nc.scalar if (w == 0 and cfg.get("w0_nfe_sc", False))
                          else nc.sync)
                    if w == 0 and cfg.get("w0_split", False):
                        hs = slots // 2
                        nq.dma_start(nfe[:, 0:hs], ZFE[:, 0:hs])
                        nq.dma_start(nfe[:, hs:slots], ZFE[:, hs:slots])
                    else:
                        nq.dma_start(nfe[:],
                                     ZFE[:, w * slots:(w + 1) * slots])
                if use_dr:
                    if w % 2 == 0:
                        dq = (nc.scalar if (w == 0 and cfg.get("w0_dfe_sc",
                                                               False))
                              else nc.sync)
                        with (tc.high_priority()
                              if w == 0 and cfg.get("w0_prio", True)
                              else contextlib.nullcontext()):
                            dfep = fpool.tile([128, 2, slots], dt.float8e4,
                                              tag="dfe", name="dfep")
                            if w == 0 and cfg.get("w0_split", False):
                                hs = slots // 2
                                dq.dma_start(dfep[:, :, 0:hs],
                                             DFE8[0, :, :, 0:hs])
                                dq.dma_start(dfep[:, :, hs:slots],
                                             DFE8[0, :, :, hs:slots])
                            else:
                                dq.dma_start(dfep[:], DFE8[w // 2, :, :, :])
                    p0 = 64 * (w % 2)
                    dfe = dfep[p0:p0 + 64, :, :]
                    wnjv = wnj2_s[p0:p0 + 64, :, :]
                else:
                    dfe = fpool.tile([128, slots], dt.float8e4, tag="dfe")
                    nc.sync.dma_start(dfe[:],
                                      DFE8[:, w * slots:(w + 1) * slots])
                ohw = mpool.tile([128, t_w, 128], dt.float8e4, tag="ohw")
                getattr(nc, cfg.get("ohw_q", "sync")).dma_start(
                    ohw[:], OHM[:, w, :, :])

                P = psP.tile([128, PW], dt.float32, tag="P")
                rhp = rpool.tile([128, t_w, PW], dt.bfloat16, tag="rhp")
                eat = lpool.tile([128, t_w, H], dt.bfloat16, tag="eat")
                wes = lpool.tile([128, t_w, H], dt.bfloat16, tag="wes")

                prPs = {}
                for hf in range(2):
                    ts0 = hf * t_half

                    def emit_pay(ts0=ts0, hf=hf):
                        for ci in range(t_half // TCH):
                            ch = hf * (t_half // TCH) + ci
                            prP = psPay.tile([128, TCH, NPAY], dt.float32,
                                             tag="prP", name="prP")
                            for k in range(TCH):
                                tl = ci * TCH + k
                                c0 = (ts0 + tl) * 128
                                nc.tensor.matmul(
                                    prP[:, k, :], lhsT=nfe[:, c0:c0 + 128],
                                    rhs=call_s[:, FE:NPROJ], start=True,
                                    stop=True, skip_group_check=True)
                            prPs[ch] = prP

                    if w == 0 and cfg.get("w0_payfirst", False):
                        emit_pay()
                    prL = psL.tile([128, t_half, FE], dt.float32, tag="prL")
                    for tl in range(t_half):
                        t = ts0 + tl
                        c0 = t * 128
                        nc.tensor.matmul(
                            prL[:, tl, :], lhsT=nfe[:, c0:c0 + 128],
                            rhs=call_s[:, 0:FE], start=True, stop=False,
                            skip_group_check=True)
                        if use_dr:
                            nc.tensor.matmul(
                                prL[:, tl, :], lhsT=dfe[:, :, c0:c0 + 128],
                                rhs=wnjv, start=False, stop=True,
                                perf_mode=PM.DoubleRow, skip_group_check=True)
                        else:
                            nc.tensor.matmul(
                                prL[:, tl, :], lhsT=dfe[:, c0:c0 + 128],
                                rhs=wnjf_s[:, 0:FE], start=False, stop=True,
                                skip_group_check=True)
                    # fused relu+attn on DVE (reads PSUM), then head reduce
                    ea = lpool.tile([128, t_half, FE], dt.bfloat16, tag="ea")
                    import contextlib as _ctl
                    with (tc.high_priority(offset=cfg.get("stt_prio", 0))
                          if cfg.get("stt_prio", 0) else _ctl.nullcontext()):
                        nc.vector.scalar_tensor_tensor(
                            out=ea[:], in0=prL[:], scalar=0.0,
                            in1=attn_s.unsqueeze(1).broadcast_to(
                                [128, t_half, FE]),
                            op0=OP.max, op1=OP.mult)
                    # attn-dot reduce of 16 as a 4-level add tree:
                    # lvl1-2 on Pool, lvl3-4 on DVE (2x-capable widths)
                    eav = ea[:].rearrange("p t (h f) -> p t h f", f=OUT_EDGE)
                    tr1 = lpool.tile([128, t_half, H, 8], dt.bfloat16,
                                     tag="tr1")
                    te12 = (nc.vector if w >= W - cfg.get("tail_tree", 0)
                            else nc.gpsimd)
                    te12.tensor_tensor(out=tr1[:], in0=eav[:, :, :, 0:8],
                                       in1=eav[:, :, :, 8:16], op=OP.add)
                    tr2 = lpool.tile([128, t_half, H, 4], dt.bfloat16,
                                     tag="tr2")
                    te12.tensor_tensor(out=tr2[:], in0=tr1[:, :, :, 0:4],
                                       in1=tr1[:, :, :, 4:8], op=OP.add)
                    tr3 = lpool.tile([128, t_half, H, 2], dt.bfloat16,
                                     tag="tr3")
                    e34 = getattr(nc, cfg.get("tree34", "gpsimd"))
                    e34.tensor_tensor(out=tr3[:], in0=tr2[:, :, :, 0:2],
                                      in1=tr2[:, :, :, 2:4], op=OP.add)
                    e34.tensor_tensor(
                        out=eat[:, ts0:ts0 + t_half, :],
                        in0=tr3[:, :, :, 0], in1=tr3[:, :, :, 1], op=OP.add)
                    if cfg.get("exp_half", True) or (
                            w >= W - cfg.get("tail_eh", 0)):
                        nc.scalar.activation(wes[:, ts0:ts0 + t_half, :],
                                             eat[:, ts0:ts0 + t_half, :],
                                             AF.Exp)
                    if not (w == 0 and cfg.get("w0_payfirst", False)):
                        emit_pay()

                # ---- weights: we = exp(eat) -------------------------------
                if not (cfg.get("exp_half", True)
                        or w >= W - cfg.get("tail_eh", 0)):
                    nc.scalar.activation(wes[:], eat[:], AF.Exp)

                # ---- payload x weight per chunk ---------------------------
                nc.gpsimd.tensor_scalar(
                    out=rhp[:, :, NPAY:PW], in0=wes[:],
                    scalar1=1.0, scalar2=None, op0=OP.mult)
                chasg = cfg.get("chasg", None)
                if chasg and w >= W - cfg.get("tailv", 0):
                    chasg = "v" * n_ch
                for ch in range(n_ch):
                    t0 = ch * TCH
                    w_rep = (wes[:, t0:t0 + TCH, :].unsqueeze(3)
                             .broadcast_to([128, TCH, H, OUT_NODE]))
                    out_v = (rhp[:, t0:t0 + TCH, 0:NPAY]
                             .rearrange("p t (h f) -> p t h f", f=OUT_NODE))
                    kind = (chasg[ch] if chasg
                            else ("v" if ch in fused_chunks else "a"))
                    if kind == "v":
                        # fused crossing+mult from psum on DVE
                        nc.vector.tensor_tensor(
                            out=out_v,
                            in0=prPs[ch][:, :, :].rearrange(
                                "p t (h f) -> p t h f", f=OUT_NODE),
                            in1=w_rep, op=OP.mult)
                    else:
                        stg = spool.tile([128, TCH, NPAY], dt.bfloat16,
                                         tag="stg")
                        nc.scalar.copy(stg[:], prPs[ch][:, :, :])
                        eng = nc.gpsimd if kind == "a" else nc.vector
                        eng.tensor_tensor(
                            out=out_v,
                            in0=stg[:].rearrange("p t (h f) -> p t h f",
                                                 f=OUT_NODE),
                            in1=w_rep, op=OP.mult)

                # ---- scatter ---------------------------------------------
                for t in range(t_w):
                    nc.tensor.matmul(P[:], lhsT=ohw[:, t, :],
                                     rhs=rhp[:, t, :],
                                     start=(t == 0), stop=(t == t_w - 1),
                                     skip_group_check=True)

                # ---- epilogue --------------------------------------------
                pb = epool.tile([128, PW], dt.bfloat16, tag="pb")
                pbe = cfg.get("pb_eng", "vector")
                if w >= W - cfg.get("tail_pb", 0):
                    pbe = "vector"
                with tc.high_priority(offset=cfg.get("prio", 300)):
                    if pbe == "vector":
                        nc.vector.tensor_scalar(
                            out=pb[:], in0=P[:], scalar1=0.0, scalar2=None,
                            op0=OP.add)
                    else:
                        nc.scalar.copy(pb[:], P[:])
                sg = epool.tile([128, H, 1], dt.float32, tag="sg")
                with tc.high_priority(offset=cfg.get("prio", 300)):
                    nc.vector.tensor_scalar(
                        out=sg[:],
                        in0=P[:, NPAY:PW].rearrange("p (h b) -> p h b", b=1),
                        scalar1=1e-30, scalar2=None, op0=OP.max)
                si = epool.tile([128, H, 1], dt.float32, tag="si")
                nc.vector.reciprocal(si[:], sg[:])
                tmp = epool.tile([128, H, OUT_NODE], dt.bfloat16, tag="tmp")
                tmp_eng = (nc.vector
                           if w >= W - cfg.get("tail_dve", 0) else nc.gpsimd)
                tmp_eng.tensor_tensor(
                    out=tmp[:],
                    in0=pb[:, 0:NPAY].rearrange("p (h f) -> p h f",
                                                f=OUT_NODE),
                    in1=si[:].broadcast_to([128, H, OUT_NODE]),
                    op=OP.mult)
                ob = w % OB
                if ob == 0:
                    outf = epool.tile([128, OB, OUT_NODE], dt.bfloat16,
                                      tag="outf", name="outf")
                    outf_cur = outf
                with nc.allow_low_precision("4-term head mean; output is "
                                            "bf16 anyway"):
                    nc.vector.tensor_reduce(
                        outf_cur[:, ob, :].unsqueeze(2),
                        tmp[:].rearrange("p h f -> p f h"),
                        axis=mybir.AxisListType.X, op=OP.add)
                if ob == OB - 1 or w == W - 1:
                    # bias + relu:  relu(x + b) == max(x, -b) + b
                    nb = ob + 1
                    nc.vector.tensor_tensor(
                        out=outf_cur[:, 0:nb, :], in0=outf_cur[:, 0:nb, :],
                        in1=negb_s.unsqueeze(1).broadcast_to(
                            [128, nb, OUT_NODE]), op=OP.max)
                    nc.vector.tensor_tensor(
                        out=outf_cur[:, 0:nb, :], in0=outf_cur[:, 0:nb, :],
                        in1=bmean_s.unsqueeze(1).broadcast_to(
                            [128, nb, OUT_NODE]), op=OP.add)
                    w0 = w - ob
                    getattr(nc, cfg.get("out_q", "scalar")).dma_start(
                        OUT[:, w0:w + 1, :], outf_cur[:, 0:nb, :])

    if not nc.is_finalized():
        nc.finalize()
    return nc


# ===========================================================================
# numpy emulation of the device program (for validation/debug)
# ===========================================================================

def emulate_core(in_map, W, cfg):
    t_half = cfg["t_half"]
    slots = 2 * t_half * 128

    f32 = np.float32
    consts = in_map["consts"].astype(f32)
    wcat = consts[:, 0:NPROJ]
    wnj2 = in_map["wnj2"].astype(f32)
    if wnj2.ndim == 3:
        wnj = wnj2.transpose(1, 0, 2).reshape(IN_NODE, FE)
    else:
        wnj = wnj2
    attn_rep = consts[0, NPROJ:NPROJ + FE]
    bmean = consts[0, NPROJ + FE:NPROJ + FE + OUT_NODE]

    out = np.zeros((W * 128, OUT_NODE), f32)
    for w in range(W):
        nfe = in_map["zfe"][:, w * slots:(w + 1) * slots].astype(f32).T
        d8 = in_map["dfe8"]
        if d8.ndim == 4:
            p0 = 64 * (w % 2)
            dfe = (d8[w // 2, p0:p0 + 64].astype(f32).transpose(1, 0, 2)
                   .reshape(IN_NODE, slots).T)
        else:
            dfe = d8[:, w * slots:(w + 1) * slots].astype(f32).T
        pay = (nfe @ wcat[:, FE:NPROJ]).astype(BF16).astype(f32)
        fout = (nfe @ wcat[:, 0:FE] + dfe @ wnj[:, 0:FE])
        r = (np.maximum(fout, 0.0) * attn_rep[None, :]).astype(BF16).astype(f32)
        eat = (r.reshape(-1, H, OUT_EDGE).sum(axis=2)).astype(BF16).astype(f32)
        wgt = np.exp(eat).astype(BF16).astype(f32)            # [slots, H]
        oh = (in_map["ohm"][:, w].astype(f32).transpose(1, 0, 2)
              .reshape(slots, 128))
        rhs = np.concatenate(
            [(pay.reshape(-1, H, OUT_NODE)
              * wgt[:, :, None]).reshape(-1, NPAY).astype(BF16).astype(f32),
             wgt], axis=1)
        P = oh.T @ rhs                                        # [128, 260]
        pb = P.astype(BF16).astype(f32)
        s = np.maximum(P[:, NPAY:PW], 1e-30)
        tmp = (pb[:, 0:NPAY].reshape(128, H, OUT_NODE)
               / s[:, :, None]).astype(BF16).astype(f32)
        acc = tmp.sum(axis=1).astype(BF16).astype(f32)
        res = np.maximum(acc, -bmean[None, :]) + bmean[None, :]
        out[w * 128:(w + 1) * 128] = res.astype(BF16).astype(f32)
    return out


def assemble(meta, results):
    n_dst = meta["cfg"]["n_dst"]
    out = np.zeros((n_dst, OUT_NODE), np.float32)
    for c in range(N_CORES):
        slots_rows, glob_rows = meta["asm"][c]
        if len(glob_rows):
            flat = (results[c]["out"].astype(np.float32)
                    .transpose(1, 0, 2).reshape(-1, OUT_NODE))
            out[glob_rows] = flat[slots_rows]
    return out


# ===========================================================================
# entry point
# ===========================================================================

_CACHE = {}
LAST_EXEC_NS = None
LAST_RESULT = None


def kernel(nfeats, dst_feats, reward, src, dst,
           W_ns, b_ns, W_ni, W_nj, W_fij, attn, b_e):
    global LAST_EXEC_NS, LAST_RESULT
    import os
    from concourse.bass_utils import run_bass_kernel_spmd

    meta, in_maps = prep(nfeats, dst_feats, reward, src, dst,
                         W_ns, b_ns, W_ni, W_nj, W_fij, attn, b_e)
    key = meta["W"]
    if key not in _CACHE:
        _CACHE[key] = build_program(meta["W"], meta["cfg"])
    nc = _CACHE[key]
    kwargs = {}
    if os.environ.get("EGAT_TRACE"):
        kwargs = dict(trace=True)
    try:
        res = run_bass_kernel_spmd(nc, in_maps, list(range(N_CORES)), **kwargs)
    except ModuleNotFoundError:
        res = run_bass_kernel_spmd(nc, in_maps, list(range(N_CORES)))
    LAST_EXEC_NS = res.exec_time_ns
    LAST_RESULT = res
    return assemble(meta, res.results)


def estimate_ns(W=None, cfg=None):
    """Cost-model (no_exec CoreSim) estimate of the per-core kernel time.

    Always builds a fresh program: sharing an nc between CoreSim and a
    real run corrupts both (the run inflates the estimate, and a prior
    estimate breaks the subsequent compile).
    """
    from concourse.bass_interp import CoreSim
    cfg = cfg or default_cfg()
    if W is None:
        W = sorted(_CACHE)[0] if _CACHE else 50
    nc = build_program(W, cfg)
    sim = CoreSim(nc, no_exec=True, publish_trace=False)
    sim.simulate()
    return int(sim.time)


# revision 44
# speedup vs baseline: 1.0008x; 1.0003x over previous
"""EGAT kernel v2 for 8 Trainium2 NeuronCores.

Edge-parallel windowed design: edges sorted by dst, 8 dst-disjoint shards,
~50 windows/core of 2048 edge slots spanning <=128 dst rows.  Per window:
project gathered src/dst features to logits and payload, softmax weights
via exp of the attn dot, weight the payload, and aggregate per dst row
with a one-hot scatter matmul.  v2 rebalance vs the v1.5 baseline:

- dst-side logit projection is one fp8 DoubleRow matmul per tile (split-K
  over channel halves): 32 cyc/tile vs 64.  dfe tiles of consecutive
  window pairs share one full-width DMA ([ceil(W/2), 128, 2, slots];
  even window on partitions 0:64, odd on 64:128, wnj2 duplicated per
  half) -- the DMA cost model charges per-partition bytes, so a
  64-partition transfer would pay double.
- relu+attn fused into one DVE scalar_tensor_tensor per half window
  (replaces the ACT relu cross + Pool attn multiply).
- the 0.01-slope lin term of leaky_relu is dropped: its logit
  contribution sits below bf16 rounding of eat (verified vs reference).
- attn-dot reduce of 16 runs as a 4-level add tree (lvl1-2 Pool,
  lvl3-4 Pool) instead of one DVE tensor_reduce.
- one merged exp per window; its [128,t,4] bf16 output feeds the scatter
  weight columns (copied into rhp) and the payload multiplies directly.
- payload PSUM crossing in 8 chunks of 2 tiles: per-chunk engine
  pattern "aavaavva" (a = ACT-stage + Pool-mult, v = DVE fused mult
  from PSUM).
- scatter is a single 260-col matmul per tile (payload + weight sums).
- epilogue: ACT crosses P once; sg/recip on DVE in f32; si-multiply on
  Pool; head-mean reduce + batched bias/relu on DVE; output DMA per 3
  windows into a [128, W, 64] layout.
- bf16 consts ride one [128, 512] DMA; all DMAs on the SP queue.

PSUM banks: logit pool 3 bufs, payload-chunk pool 3, scatter pool 2
(the third logit buffer relieves the prL-recycle stall on the PE).

Cost-model estimate 218174 ns/core (baseline 282281); HW-verified
rel err 0.0051 (gate 2e-2).
"""

import sys

sys.path.insert(0, "/opt/trn_rl_repo")

import numpy as np
import ml_dtypes

BF16 = ml_dtypes.bfloat16
FP8 = ml_dtypes.float8_e4m3

# ---- problem constants (hardcoded per the task contract) -------------------
N_SRC = 50000
N_DST = 50000
E = 800000
IN_NODE = 128
IN_EDGE = 16
OUT_NODE = 64
OUT_EDGE = 16
H = 4
SLOPE = 0.01

N_CORES = 8

FE = H * OUT_EDGE            # 64 logit cols
NPAY = H * OUT_NODE          # 256 payload cols
NPROJ = FE + NPAY            # 320
PW = NPAY + H                # 260 scatter cols (payload + 4 weight sums)
LSCALE = 8.0                 # logit-projection scale (fp8 subnormal dodge)


def default_cfg():
    return dict(
        n_dst=N_DST,
        t_half=8,             # tiles per half-window (8 -> 1024 slots/half)
        span=128,             # max dst rows per window
        use_dr=True,          # fp8 DoubleRow for dst-side logit matmuls
        ohw_q="sync",         # DMA queue for the one-hot
        pb_eng="scalar",      # engine for the P psum->sbuf cross
        tree34="gpsimd",      # engine for eat-tree levels 3-4
        prio=300,             # priority boost for P_ext-releasing ops
        tch=2,                # tiles per payload psum chunk
        chasg="aavaavva",     # per-chunk engine: a=ACT+Pool, v=DVE-fused
        out_q="sync",         # DMA queue for the output
        ob=3,                 # windows per output DMA batch
        exp_half=False,       # one merged exp per window
        w0_prio=False,        # priority boost for window-0 DMAs
        cq="sync",            # DMA queue for one-time consts
        tailv=0,              # trailing windows with all-DVE chunks
        tail_dve=1,           # final windows: si-multiply on DVE (short chain)
        tail_pb=1,            # final windows: P-cross on DVE (short chain)
        tail_eh=2,            # final windows: exp per half (short chain)
        psl=3,                # PSUM bufs for the logit pool
        pspay=3,              # PSUM bufs for the payload-chunk pool
    )


# ===========================================================================
# Host-side packing
# ===========================================================================

def prep(nfeats, dst_feats, reward, src, dst, W_ns, b_ns, W_ni, W_nj, W_fij,
         attn, b_e, cfg=None):
    """Sort/shard/pack everything. Returns (meta, in_maps)."""
    cfg = cfg or default_cfg()
    n_dst = cfg["n_dst"]
    t_half = cfg["t_half"]
    span = cfg["span"]
    slots = 2 * t_half * 128    # slots per window
    t_w = 2 * t_half

    e_tot = src.shape[0]

    nfeats = np.asarray(nfeats, np.float32)
    dst_feats = np.asarray(dst_feats, np.float32)
    reward = np.asarray(reward, np.float32)
    src = np.asarray(src, np.int64)
    dst = np.asarray(dst, np.int64)
    W_ns = np.asarray(W_ns, np.float32)
    b_ns = np.asarray(b_ns, np.float32)
    W_ni = np.asarray(W_ni, np.float32)
    W_nj = np.asarray(W_nj, np.float32)
    W_fij = np.asarray(W_fij, np.float32)
    attn = np.asarray(attn, np.float32)
    b_e = np.asarray(b_e, np.float32)

    # ---- sort by dst and shard at dst boundaries --------------------------
    order = np.argsort(dst, kind="stable")
    d_s = dst[order]
    s_s = src[order]
    r_s = reward[order]

    cut = [0]
    for c in range(1, N_CORES):
        t = (e_tot * c) // N_CORES
        while t < e_tot and t > 0 and d_s[t] == d_s[t - 1]:
            t += 1
        cut.append(t)
    cut.append(e_tot)

    # ---- greedy window packing per core -----------------------------------
    per_core = []
    for c in range(N_CORES):
        e0, e1 = cut[c], cut[c + 1]
        d = d_s[e0:e1]
        wins = []  # (base, w_start, w_count) over local positions
        if e1 > e0:
            uniq, starts = np.unique(d, return_index=True)
            ends = np.append(starts[1:], len(d))
            base = None
            w_start = 0
            w_count = 0
            for gi in range(len(uniq)):
                dd = int(uniq[gi])
                glen = int(ends[gi] - starts[gi])
                if (base is None or dd - base > span - 1
                        or w_count + glen > slots):
                    if base is not None:
                        wins.append((base, w_start, w_count))
                    base = dd
                    w_start = int(starts[gi])
                    w_count = 0
                w_count += glen
            wins.append((base, w_start, w_count))
        per_core.append((e0, e1, wins))

    W = max(1, max(len(pc[2]) for pc in per_core))

    # virtual feature rows: x_row @ W_nj == colsum(W_fij); y_row @ W_nj == b_e
    wsum = W_fij.sum(axis=0)
    x_row = np.linalg.lstsq(W_nj.T.astype(np.float64), wsum.astype(np.float64),
                            rcond=None)[0].astype(np.float32)
    y_row = np.linalg.lstsq(W_nj.T.astype(np.float64), b_e.astype(np.float64),
                            rcond=None)[0].astype(np.float32)

    mf_all = []     # [128, W, t_w, 128] fp8 one-hot per slot
    zfe_all = []    # per core (zfe bf16 [128, W*slots], dfe8 [64, 2, W*slots])
    asm = []        # per core (slot_rows, global_rows)

    for c in range(N_CORES):
        e0, e1, wins = per_core[c]
        d = d_s[e0:e1]
        s = s_s[e0:e1]
        r = r_s[e0:e1]

        drel = np.full((W, slots), -1.0, np.float32)
        nfe = np.zeros((W * slots, IN_NODE), np.float32)
        dfe = np.zeros((W * slots, IN_NODE), np.float32)
        rows_slot = []
        rows_glob = []
        for w, (base, ws, wc) in enumerate(wins):
            sl = slice(ws, ws + wc)
            drel[w, :wc] = (d[sl] - base).astype(np.float32)
            nfe[w * slots:w * slots + wc] = nfeats[s[sl]]
            dfe[w * slots:w * slots + wc] = (dst_feats[d[sl]]
                                             + r[sl, None] * x_row[None, :]
                                             + y_row[None, :])
            uds = np.unique(d[sl])
            rows_slot.append(w * 128 + (uds - base))
            rows_glob.append(uds)

        # one-hot per slot, layout [128 p, W, t, 128 dcol]
        ohm = (drel.reshape(W, t_w, 128)[:, :, :, None]
               == np.arange(128, dtype=np.float32)).astype(FP8)
        ohm = np.ascontiguousarray(ohm.transpose(2, 0, 1, 3))

        zfe = np.ascontiguousarray(
            nfe.T.reshape(IN_NODE, W * slots).astype(BF16))
        if cfg.get("use_dr", True):
            # dfe channel-split for DoubleRow, window-pair packed:
            # [ceil(W/2), 128, 2, slots] -- even window on partitions 0:64,
            # odd window on 64:128, so each DMA moves a full-width tile.
            d4 = dfe.reshape(W, slots, 2, 64).transpose(0, 3, 2, 1).astype(FP8)
            if W % 2:
                d4 = np.concatenate(
                    [d4, np.zeros_like(d4[:1])], axis=0)
            df8 = np.ascontiguousarray(
                np.concatenate([d4[0::2], d4[1::2]], axis=1))
        else:
            df8 = np.ascontiguousarray(
                dfe.T.reshape(IN_NODE, W * slots).astype(FP8))
        mf_all.append(ohm)
        zfe_all.append((zfe, df8))
        asm.append((np.concatenate(rows_slot) if rows_slot else
                    np.zeros(0, np.int64),
                    np.concatenate(rows_glob) if rows_glob else
                    np.zeros(0, np.int64)))

    # ---- shared constants -------------------------------------------------
    # e = sum_f attn*leaky(x) ~= 0.99*sum_f attn*relu(x)  (relu
    # decomposition of leaky_relu; the 0.99 is folded into attn_rep and
    # the 0.01-slope lin term is dropped -- its logit contribution sits
    # below bf16 rounding of eat, verified vs the reference).
    wcat = np.concatenate([W_ni * LSCALE, W_ns * 0.25],
                          axis=1).astype(BF16)                       # [128,320]
    wnj_full = W_nj * LSCALE
    if cfg.get("use_dr", True):
        w2 = wnj_full.reshape(2, 64, FE).transpose(1, 0, 2).astype(FP8)
        wnj2 = np.ascontiguousarray(np.concatenate([w2, w2], axis=0))
    else:
        wnj2 = np.ascontiguousarray(wnj_full.astype(FP8))
    attn_rep = np.broadcast_to(
        (0.99 / LSCALE) * attn.reshape(-1).astype(np.float32),
        (128, H * OUT_EDGE)).astype(BF16).copy()
    bmean = np.broadcast_to(b_ns.reshape(H, OUT_NODE).mean(axis=0),
                            (128, OUT_NODE)).astype(BF16).copy()
    negb = (-bmean.astype(np.float32)).astype(BF16)

    consts = np.concatenate(
        [wcat, attn_rep, bmean, negb], axis=1)                   # [128, 512]
    in_maps = []
    for c in range(N_CORES):
        in_maps.append(dict(
            zfe=zfe_all[c][0], dfe8=zfe_all[c][1], ohm=mf_all[c],
            consts=consts, wnj2=wnj2,
        ))

    meta = dict(W=W, asm=asm, cfg=cfg)
    return meta, in_maps


# ===========================================================================
# Device program
# ===========================================================================

def build_program(W, cfg, fused_chunks=None):
    import concourse.bacc as bacc
    import concourse.tile as tile
    import concourse.mybir as mybir
    from contextlib import ExitStack

    dt = mybir.dt
    AF = mybir.ActivationFunctionType
    OP = mybir.AluOpType
    PM = mybir.MatmulPerfMode

    if fused_chunks is None:
        fused_chunks = tuple(cfg.get("fused", (1,)))
    t_half = cfg["t_half"]
    t_w = 2 * t_half
    slots = t_w * 128
    TCH = cfg.get("tch", 4)      # tiles per payload psum chunk
    n_ch = t_w // TCH            # payload chunks per window
    OB = cfg.get("ob", 4)       # windows per output DMA batch

    nc = bacc.Bacc(None, target_bir_lowering=False)

    ZFE = nc.declare_dram_parameter("zfe", [IN_NODE, W * slots],
                                    dt.bfloat16, isOutput=False)
    use_dr = cfg.get("use_dr", True)
    if use_dr:
        DFE8 = nc.declare_dram_parameter("dfe8",
                                         [(W + 1) // 2, 128, 2, slots],
                                         dt.float8e4, isOutput=False)
        WNJ2 = nc.declare_dram_parameter("wnj2", [128, 2, FE], dt.float8e4,
                                         isOutput=False)
    else:
        DFE8 = nc.declare_dram_parameter("dfe8", [IN_NODE, W * slots],
                                         dt.float8e4, isOutput=False)
        WNJ2 = nc.declare_dram_parameter("wnj2", [IN_NODE, FE],
                                         dt.float8e4, isOutput=False)
    OHM = nc.declare_dram_parameter("ohm", [128, W, t_w, 128],
                                    dt.float8e4, isOutput=False)
    CONSTS = nc.declare_dram_parameter("consts", [IN_NODE, NPROJ + 3 * FE],
                                       dt.bfloat16, isOutput=False)
    OUT = nc.declare_dram_parameter("out", [128, W, OUT_NODE], dt.bfloat16,
                                    isOutput=True)

    with tile.TileContext(nc) as tc, ExitStack() as ctx:
        cpool = ctx.enter_context(tc.tile_pool(name="consts", bufs=1))
        call_s = cpool.tile([128, NPROJ + 3 * FE], dt.bfloat16)
        getattr(nc, cfg.get("cq", "scalar")).dma_start(call_s[:], CONSTS[:])
        wcat_s = call_s[:, 0:NPROJ]
        attn_s = call_s[:, NPROJ:NPROJ + FE]
        bmean_s = call_s[:, NPROJ + FE:NPROJ + FE + OUT_NODE]
        negb_s = call_s[:, NPROJ + FE + OUT_NODE:NPROJ + FE + 2 * OUT_NODE]
        if use_dr:
            wnj2_s = cpool.tile([128, 2, FE], dt.float8e4)
            getattr(nc, cfg.get("cq", "scalar")).dma_start(wnj2_s[:], WNJ2[:])
        else:
            wnjf_s = cpool.tile([128, FE], dt.float8e4)
            nc.sync.dma_start(wnjf_s[:], WNJ2[:])

        BUFS = cfg.get("bufs", (5, 5, 5, 8, 4, 6))
        with tc.tile_pool(name="feat", bufs=BUFS[0]) as fpool, \
             tc.tile_pool(name="meta", bufs=BUFS[1]) as mpool, \
             tc.tile_pool(name="lrp", bufs=BUFS[2]) as lpool, \
             tc.tile_pool(name="stgp", bufs=BUFS[3]) as spool, \
             tc.tile_pool(name="rhsp", bufs=BUFS[4]) as rpool, \
             tc.tile_pool(name="ep", bufs=BUFS[5]) as epool, \
             tc.tile_pool(name="psL", bufs=cfg.get("psl", 2),
                          space="PSUM") as psL, \
             tc.tile_pool(name="psPay", bufs=cfg.get("pspay", 8 // TCH),
                          space="PSUM") as psPay, \
             tc.tile_pool(name="psP", bufs=cfg.get("psp", 2),
                          space="PSUM") as psP:
            for w in range(W):
                import contextlib
                prio_ctx = (tc.high_priority()
                            if w == 0 and cfg.get("w0_prio", True)
                            else contextlib.nullcontext())
                with prio_ctx:
                    nfe = fpool.tile([128, slots], dt.bfloat16, tag="nfe")
                    nq = (nc.scalar if (w == 0 and cfg.get("w0_nfe_sc", False))
                          else nc.sync)
                    if w == 0 and cfg.get("w0_split", False):
                        hs = slots // 2
                        nq.dma_start(nfe[:, 0:hs], ZFE[:, 0:hs])
                        nq.dma_start(nfe[:, hs:slots], ZFE[:, hs:slots])
                    else:
                        nq.dma_start(nfe[:],
                                     ZFE[:, w * slots:(w + 1) * slots])
                if use_dr:
                    if w % 2 == 0:
                        dq = (nc.scalar if (w == 0 and cfg.get("w0_dfe_sc",
                                                               False))
                              else nc.sync)
                        with (tc.high_priority()
                              if w == 0 and cfg.get("w0_prio", True)
                              else contextlib.nullcontext()):
                            dfep = fpool.tile([128, 2, slots], dt.float8e4,
                                              tag="dfe", name="dfep")
                            if w == 0 and cfg.get("w0_split", False):
                                hs = slots // 2
                                dq.dma_start(dfep[:, :, 0:hs],
                                             DFE8[0, :, :, 0:hs])
                                dq.dma_start(dfep[:, :, hs:slots],
                                             DFE8[0, :, :, hs:slots])
                            else:
                                dq.dma_start(dfep[:], DFE8[w // 2, :, :, :])
                    p0 = 64 * (w % 2)
                    dfe = dfep[p0:p0 + 64, :, :]
                    wnjv = wnj2_s[p0:p0 + 64, :, :]
                else:
                    dfe = fpool.tile([128, slots], dt.float8e4, tag="dfe")
                    nc.sync.dma_start(dfe[:],
                                      DFE8[:, w * slots:(w + 1) * slots])
                ohw = mpool.tile([128, t_w, 128], dt.float8e4, tag="ohw")
                getattr(nc, cfg.get("ohw_q", "sync")).dma_start(
                    ohw[:], OHM[:, w, :, :])

                P = psP.tile([128, PW], dt.float32, tag="P")
                rhp = rpool.tile([128, t_w, PW], dt.bfloat16, tag="rhp")
                eat = lpool.tile([128, t_w, H], dt.bfloat16, tag="eat")
                wes = lpool.tile([128, t_w, H], dt.bfloat16, tag="wes")

                prPs = {}
                for hf in range(2):
                    ts0 = hf * t_half

                    def emit_pay(ts0=ts0, hf=hf):
                        for ci in range(t_half // TCH):
                            ch = hf * (t_half // TCH) + ci
                            prP = psPay.tile([128, TCH, NPAY], dt.float32,
                                             tag="prP", name="prP")
                            for k in range(TCH):
                                tl = ci * TCH + k
                                c0 = (ts0 + tl) * 128
                                nc.tensor.matmul(
                                    prP[:, k, :], lhsT=nfe[:, c0:c0 + 128],
                                    rhs=call_s[:, FE:NPROJ], start=True,
                                    stop=True, skip_group_check=True)
                            prPs[ch] = prP

                    if w == 0 and cfg.get("w0_payfirst", False):
                        emit_pay()
                    prL = psL.tile([128, t_half, FE], dt.float32, tag="prL")
                    for tl in range(t_half):
                        t = ts0 + tl
                        c0 = t * 128
                        nc.tensor.matmul(
                            prL[:, tl, :], lhsT=nfe[:, c0:c0 + 128],
                            rhs=call_s[:, 0:FE], start=True, stop=False,
                            skip_group_check=True)
                        if use_dr:
                            nc.tensor.matmul(
                                prL[:, tl, :], lhsT=dfe[:, :, c0:c0 + 128],
                                rhs=wnjv, start=False, stop=True,
                                perf_mode=PM.DoubleRow, skip_group_check=True)
                        else:
                            nc.tensor.matmul(
                                prL[:, tl, :], lhsT=dfe[:, c0:c0 + 128],
                                rhs=wnjf_s[:, 0:FE], start=False, stop=True,
                                skip_group_check=True)
                    # fused relu+attn on DVE (reads PSUM), then head reduce
                    ea = lpool.tile([128, t_half, FE], dt.bfloat16, tag="ea")
                    import contextlib as _ctl
                    with (tc.high_priority(offset=cfg.get("stt_prio", 0))
                          if cfg.get("stt_prio", 0) else _ctl.nullcontext()):
                        nc.vector.scalar_tensor_tensor(
                            out=ea[:], in0=prL[:], scalar=0.0,
                            in1=attn_s.unsqueeze(1).broadcast_to(
                                [128, t_half, FE]),
                            op0=OP.max, op1=OP.mult)
                    # attn-dot reduce of 16 as a 4-level add tree:
                    # lvl1-2 on Pool, lvl3-4 on DVE (2x-capable widths)
                    eav = ea[:].rearrange("p t (h f) -> p t h f", f=OUT_EDGE)
                    tr1 = lpool.tile([128, t_half, H, 8], dt.bfloat16,
                                     tag="tr1")
                    te12 = (nc.vector if w >= W - cfg.get("tail_tree", 0)
                            else nc.gpsimd)
                    te12.tensor_tensor(out=tr1[:], in0=eav[:, :, :, 0:8],
                                       in1=eav[:, :, :, 8:16], op=OP.add)
                    tr2 = lpool.tile([128, t_half, H, 4], dt.bfloat16,
                                     tag="tr2")
                    te12.tensor_tensor(out=tr2[:], in0=tr1[:, :, :, 0:4],
                                       in1=tr1[:, :, :, 4:8], op=OP.add)
                    tr3 = lpool.tile([128, t_half, H, 2], dt.bfloat16,
                                     tag="tr3")
                    e34 = getattr(nc, cfg.get("tree34", "gpsimd"))
                    e34.tensor_tensor(out=tr3[:], in0=tr2[:, :, :, 0:2],
                                      in1=tr2[:, :, :, 2:4], op=OP.add)
                    e34.tensor_tensor(
                        out=eat[:, ts0:ts0 + t_half, :],
                        in0=tr3[:, :, :, 0], in1=tr3[:, :, :, 1], op=OP.add)
                    if cfg.get("exp_half", True) or (
                            w >= W - cfg.get("tail_eh", 0)):
                        nc.scalar.activation(wes[:, ts0:ts0 + t_half, :],
                                             eat[:, ts0:ts0 + t_half, :],
                                             AF.Exp)
                    if not (w == 0 and cfg.get("w0_payfirst", False)):
                        emit_pay()

                # ---- weights: we = exp(eat) -------------------------------
                if not (cfg.get("exp_half", True)
                        or w >= W - cfg.get("tail_eh", 0)):
                    nc.scalar.activation(wes[:], eat[:], AF.Exp)

                # ---- payload x weight per chunk ---------------------------
                nc.gpsimd.tensor_scalar(
                    out=rhp[:, :, NPAY:PW], in0=wes[:],
                    scalar1=1.0, scalar2=None, op0=OP.mult)
                chasg = cfg.get("chasg", None)
                if chasg and w >= W - cfg.get("tailv", 0):
                    chasg = "v" * n_ch
                for ch in range(n_ch):
                    t0 = ch * TCH
                    w_rep = (wes[:, t0:t0 + TCH, :].unsqueeze(3)
                             .broadcast_to([128, TCH, H, OUT_NODE]))
                    out_v = (rhp[:, t0:t0 + TCH, 0:NPAY]
                             .rearrange("p t (h f) -> p t h f", f=OUT_NODE))
                    kind = (chasg[ch] if chasg
                            else ("v" if ch in fused_chunks else "a"))
                    if kind == "v":
                        # fused crossing+mult from psum on DVE
                        nc.vector.tensor_tensor(
                            out=out_v,
                            in0=prPs[ch][:, :, :].rearrange(
                                "p t (h f) -> p t h f", f=OUT_NODE),
                            in1=w_rep, op=OP.mult)
                    else:
                        stg = spool.tile([128, TCH, NPAY], dt.bfloat16,
                                         tag="stg")
                        nc.scalar.copy(stg[:], prPs[ch][:, :, :])
                        eng = nc.gpsimd if kind == "a" else nc.vector
                        eng.tensor_tensor(
                            out=out_v,
                            in0=stg[:].rearrange("p t (h f) -> p t h f",
                                                 f=OUT_NODE),
                            in1=w_rep, op=OP.mult)

                # ---- scatter ---------------------------------------------
                for t in range(t_w):
                    nc.tensor.matmul(P[:], lhsT=ohw[:, t, :],
                                     rhs=rhp[:, t, :],
                                     start=(t == 0), stop=(t == t_w - 1),
                                     skip_group_check=True)

                # ---- epilogue --------------------------------------------
                pb = epool.tile([128, PW], dt.bfloat16, tag="pb")
                pbe = cfg.get("pb_eng", "vector")
                if w >= W - cfg.get("tail_pb", 0):
                    pbe = "vector"
                with tc.high_priority(offset=cfg.get("prio", 300)):
                    if pbe == "vector":
                        nc.vector.tensor_scalar(
                            out=pb[:], in0=P[:], scalar1=0.0, scalar2=None,
                            op0=OP.add)
                    else:
                        nc.scalar.copy(pb[:], P[:])
                sg = epool.tile([128, H, 1], dt.float32, tag="sg")
                with tc.high_priority(offset=cfg.get("prio", 300)):
                    nc.vector.tensor_scalar(
                        out=sg[:],
                        in0=P[:, NPAY:PW].rearrange("p (h b) -> p h b", b=1),
                        scalar1=1e-30, scalar2=None, op0=OP.max)
                si = epool.tile([128, H, 1], dt.float32, tag="si")
                nc.vector.reciprocal(si[:], sg[:])
                tmp = epool.tile([128, H, OUT_NODE], dt.bfloat16, tag="tmp")
                tmp_eng = (nc.vector
                           if w >= W - cfg.get("tail_dve", 0) else nc.gpsimd)
                tmp_eng.tensor_tensor(
                    out=tmp[:],
                    in0=pb[:, 0:NPAY].rearrange("p (h f) -> p h f",
                                                f=OUT_NODE),
                    in1=si[:].broadcast_to([128, H, OUT_NODE]),
                    op=OP.mult)
                ob = w % OB
                if ob == 0:
                    outf = epool.tile([128, OB, OUT_NODE], dt.bfloat16,
                                      tag="outf", name="outf")
                    outf_cur = outf
                with nc.allow_low_precision("4-term head mean; output is "
                                            "bf16 anyway"):
                    nc.vector.tensor_reduce(
                        outf_cur[:, ob, :].unsqueeze(2),
                        tmp[:].rearrange("p h f -> p f h"),
                        axis=mybir.AxisListType.X, op=OP.add)
                if ob == OB - 1 or w == W - 1:
                    # bias + relu:  relu(x + b) == max(x, -b) + b
                    nb = ob + 1
                    nc.vector.tensor_tensor(
                        out=outf_cur[:, 0:nb, :], in0=outf_cur[:, 0:nb, :],
                        in1=negb_s.unsqueeze(1).broadcast_to(
                            [128, nb, OUT_NODE]), op=OP.max)
                    nc.vector.tensor_tensor(
                        out=outf_cur[:, 0:nb, :], in0=outf_cur[:, 0:nb, :],
                        in1=bmean_s.unsqueeze(1).broadcast_to(
                            [128, nb, OUT_NODE]), op=OP.add)
                    w0 = w - ob
                    getattr(nc, cfg.get("out_q", "scalar")).dma_start(
                        OUT[:, w0:w + 1, :], outf_cur[:, 0:nb, :])

    if not nc.is_finalized():
        nc.finalize()
    return nc


# ===========================================================================
# numpy emulation of the device program (for validation/debug)
# ===========================================================================

def emulate_core(in_map, W, cfg):
    t_half = cfg["t_half"]
    slots = 2 * t_half * 128

    f32 = np.float32
    consts = in_map["consts"].astype(f32)
    wcat = consts[:, 0:NPROJ]
    wnj2 = in_map["wnj2"].astype(f32)
    if wnj2.ndim == 3:
        wnj = wnj2.transpose(1, 0, 2).reshape(IN_NODE, FE)
    else:
        wnj = wnj2
    attn_rep = consts[0, NPROJ:NPROJ + FE]
    bmean = consts[0, NPROJ + FE:NPROJ + FE + OUT_NODE]

    out = np.zeros((W * 128, OUT_NODE), f32)
    for w in range(W):
        nfe = in_map["zfe"][:, w * slots:(w + 1) * slots].astype(f32).T
        d8 = in_map["dfe8"]
        if d8.ndim == 4:
            p0 = 64 * (w % 2)
            dfe = (d8[w // 2, p0:p0 + 64].astype(f32).transpose(1, 0, 2)
                   .reshape(IN_NODE, slots).T)
        else:
            dfe = d8[:, w * slots:(w + 1) * slots].astype(f32).T
        pay = (nfe @ wcat[:, FE:NPROJ]).astype(BF16).astype(f32)
        fout = (nfe @ wcat[:, 0:FE] + dfe @ wnj[:, 0:FE])
        r = (np.maximum(fout, 0.0) * attn_rep[None, :]).astype(BF16).astype(f32)
        eat = (r.reshape(-1, H, OUT_EDGE).sum(axis=2)).astype(BF16).astype(f32)
        wgt = np.exp(eat).astype(BF16).astype(f32)            # [slots, H]
        oh = (in_map["ohm"][:, w].astype(f32).transpose(1, 0, 2)
              .reshape(slots, 128))
        rhs = np.concatenate(
            [(pay.reshape(-1, H, OUT_NODE)
              * wgt[:, :, None]).reshape(-1, NPAY).astype(BF16).astype(f32),
             wgt], axis=1)
        P = oh.T @ rhs                                        # [128, 260]
        pb = P.astype(BF16).astype(f32)
        s = np.maximum(P[:, NPAY:PW], 1e-30)
        tmp = (pb[:, 0:NPAY].reshape(128, H, OUT_NODE)
               / s[:, :, None]).astype(BF16).astype(f32)
        acc = tmp.sum(axis=1).astype(BF16).astype(f32)
        res = np.maximum(acc, -bmean[None, :]) + bmean[None, :]
        out[w * 128:(w + 1) * 128] = res.astype(BF16).astype(f32)
    return out


def assemble(meta, results):
    n_dst = meta["cfg"]["n_dst"]
    out = np.zeros((n_dst, OUT_NODE), np.float32)
    for c in range(N_CORES):
        slots_rows, glob_rows = meta["asm"][c]
        if len(glob_rows):
            flat = (results[c]["out"].astype(np.float32)
                    .transpose(1, 0, 2).reshape(-1, OUT_NODE))
            out[glob_rows] = flat[slots_rows]
    return out


# ===========================================================================
# entry point
# ===========================================================================

_CACHE = {}
LAST_EXEC_NS = None
LAST_RESULT = None


def kernel(nfeats, dst_feats, reward, src, dst,
           W_ns, b_ns, W_ni, W_nj, W_fij, attn, b_e):
    global LAST_EXEC_NS, LAST_RESULT
    import os
    from concourse.bass_utils import run_bass_kernel_spmd

    meta, in_maps = prep(nfeats, dst_feats, reward, src, dst,
                         W_ns, b_ns, W_ni, W_nj, W_fij, attn, b_e)
    key = meta["W"]
    if key not in _CACHE:
        _CACHE[key] = build_program(meta["W"], meta["cfg"])
    nc = _CACHE[key]
    kwargs = {}
    if os.environ.get("EGAT_TRACE"):
        kwargs = dict(trace=True)
    try:
        res = run_bass_kernel_spmd(nc, in_maps, list(range(N_CORES)), **kwargs)
    except ModuleNotFoundError:
        res = run_bass_kernel_spmd(nc, in_maps, list(range(N_CORES)))
    LAST_EXEC_NS = res.exec_time_ns
    LAST_RESULT = res
    return assemble(meta, res.results)


def estimate_ns(W=None, cfg=None):
    """Cost-model (no_exec CoreSim) estimate of the per-core kernel time.

    Always builds a fresh program: sharing an nc between CoreSim and a
    real run corrupts both (the run inflates the estimate, and a prior
    estimate breaks the subsequent compile).
    """
    from concourse.bass_interp import CoreSim
    cfg = cfg or default_cfg()
    if W is None:
        W = sorted(_CACHE)[0] if _CACHE else 50
    nc = build_program(W, cfg)
    sim = CoreSim(nc, no_exec=True, publish_trace=False)
    sim.simulate()
    return int(sim.time)
